# revision 1
# baseline (speedup 1.0000x reference)
"""Trainium2 Bass kernel for nn_EpipolarWarpOperator (B=8, C=320, H=W=64, S=3).

Sharding: pure data parallelism — one batch element per NeuronCore (8 cores).

Per-core pipeline (fp16 on-chip storage, fp32 PSUM accumulate):
  host: epipolar geometry -> bilinear corner indices/weights; samples sorted
        by y-group; S matrix [128, 20480] (4 nnz/col, bilinear*1/3, padded to
        128-aligned groups); slab row-gather indices; unsort gather indices.
  A: slab(g_b) = x^T rows [64g, 64g+128) fetched by indexed dma_gather from
     HBM; val[128 tok, 320 ch] = S_b.T @ slab  (PE matmul per 128-token block)
  B: unsort: SBUF-source transpose dma_gather of val rows by sorted position
     (per sample s) -> channel-major [128, 3, npix]; mean over s on DVE,
     written into a zero-padded 66x66 layout.
  C: 3x3 conv as 9 shifted matmuls over (mchunk, kchunk, tap), bias+ReLU on
     ACT, DMA out.
"""

import numpy as np

B, C, H, W = 8, 320, 64, 64
NUM_SAMPLES = 3
HW = H * W
NBLK = 160            # fixed token-block budget (>= 12288 + 64*127 padded)
NPAD = NBLK * 128
CPAD = 384            # channel pad so gather elem = 768B (mult of 256)
XROWS = 4224          # x^T rows incl. pad (max slab row 4159)
PW, PH = W + 2, H + 2
MB = [(0, 128), (128, 128), (256, 64)]   # channel chunking

import os as _os
A_CH = int(_os.environ.get("K_ACH", "16"))
B_CH = int(_os.environ.get("K_BCH", "512"))
CONV_PAIR = _os.environ.get("K_PAIR", "1") == "1"
CONV_ILV = _os.environ.get("K_ILV", "0") == "1"
SCRATCH = int(_os.environ.get("K_SCRATCH", "16384"))

assert HW % B_CH == 0

# ---------------------------------------------------------------- host prep

def _rodrigues_np(rv):
    theta = np.sqrt((rv * rv).sum())
    r = rv / max(theta, 1e-12)
    I = np.eye(3, dtype=np.float32)
    K = np.array([[0, -r[2], r[1]], [r[2], 0, -r[0]], [-r[1], r[0], 0]],
                 dtype=np.float32)
    R = np.cos(theta) * I + (1 - np.cos(theta)) * np.outer(r, r) + np.sin(theta) * K
    return I if theta < 1e-6 else R


def fundamental_np(Ks, Kt, ps, pt):
    Fs = []
    for b in range(Ks.shape[0]):
        Rs = _rodrigues_np(ps[b, :3].astype(np.float32))
        Rt = _rodrigues_np(pt[b, :3].astype(np.float32))
        ts_, tt_ = ps[b, 3:].astype(np.float32), pt[b, 3:].astype(np.float32)
        R_rel = Rs @ Rt.T
        t_rel = ts_ - R_rel @ tt_
        z = np.float32(0)
        skew = np.array([[z, -t_rel[2], t_rel[1]],
                         [t_rel[2], z, -t_rel[0]],
                         [-t_rel[1], t_rel[0], z]], dtype=np.float32)
        E = skew @ R_rel
        inv_Ks = np.linalg.inv(Ks[b].astype(np.float32))
        inv_Kt = np.linalg.inv(Kt[b].astype(np.float32))
        Fs.append(inv_Kt.T @ E @ inv_Ks)
    return np.stack(Fs).astype(np.float32)


def geometry(F):
    k = np.arange(HW)
    px = (k % W).astype(np.float32)
    py = (k // W).astype(np.float32)
    P = np.stack([px, py, np.ones_like(px)])
    lines = F.T.astype(np.float32) @ P
    a, b_, c = lines[0], lines[1], lines[2]
    W1, H1 = np.float32(W - 1), np.float32(H - 1)
    EPS = np.float32(1e-10)
    x1 = np.clip(-c / (a + EPS), 0.0, W1)
    x2 = np.clip(-(b_ * H1 + c) / (a + EPS), 0.0, W1)
    y1 = np.clip(-c / (b_ + EPS), 0.0, H1)
    y2 = np.clip(-(a * W1 + c) / (b_ + EPS), 0.0, H1)
    t = np.linspace(0.0, 1.0, NUM_SAMPLES, dtype=np.float32)
    sx = x1[:, None] * (1 - t) + x2[:, None] * t
    sy = y1[:, None] * (1 - t) + y2[:, None] * t
    x0 = np.floor(sx)
    y0 = np.floor(sy)
    wx = (sx - x0).astype(np.float32)
    wy = (sy - y0).astype(np.float32)
    x0i = np.clip(x0, 0, W - 1).astype(np.int32)
    y0i = np.clip(y0, 0, H - 1).astype(np.int32)
    return x0i, y0i, wx, wy


def build_sort(x0i, y0i, wx, wy):
    """x0i/y0i/wx/wy: [npix, S] for one pixel range. Returns S weights,
    per-block slab group, per-sample padded position, used block count."""
    flat_y = y0i.reshape(-1)
    order = np.argsort(flat_y, kind='stable')
    S = np.zeros((128, NPAD), dtype=np.float32)
    pos = np.zeros(flat_y.size, dtype=np.int32)
    blk_g = np.zeros(NBLK, dtype=np.int32)
    cur = 0
    x0f = x0i.reshape(-1)
    wxf = wx.reshape(-1)
    wyf = wy.reshape(-1)
    third = np.float32(1.0 / 3.0)
    for g in range(H):
        sel = order[flat_y[order] == g]
        n = sel.size
        if n == 0:
            continue
        cols = cur + np.arange(n)
        pos[sel] = cols
        x0s = x0f[sel]
        wxs = wxf[sel]
        wys = wyf[sel]
        x1s = np.minimum(x0s + 1, W - 1)
        np.add.at(S, (x0s, cols), (1 - wys) * (1 - wxs) * third)
        np.add.at(S, (x1s, cols), (1 - wys) * wxs * third)
        np.add.at(S, (64 + x0s, cols), wys * (1 - wxs) * third)
        np.add.at(S, (64 + x1s, cols), wys * wxs * third)
        nb_lo = cur // 128
        cur = ((cur + n + 127) // 128) * 128
        blk_g[nb_lo:cur // 128] = g
    assert cur <= NPAD, cur
    npix = x0i.shape[0]
    return (S.astype(np.float16), blk_g, pos.reshape(npix, NUM_SAMPLES),
            cur // 128)


def wrap16(idx, n):
    t = idx.astype(np.int16).reshape(n // 16, 16).T
    return np.tile(t, (8, 1)).copy()


def prep_batch(xb, F, nhalves):
    """Sort each pixel range independently so the device can overlap
    phase A of range h+1 with phase B of range h."""
    x0i, y0i, wx, wy = geometry(F)
    xt = np.zeros((XROWS, CPAD), dtype=np.float16)
    xt[:HW, :C] = xb.reshape(C, HW).T.astype(np.float16)
    hp = HW // nhalves
    parts = []
    for h in range(nhalves):
        sl = slice(h * hp, (h + 1) * hp)
        parts.append(build_sort(x0i[sl], y0i[sl], wx[sl], wy[sl]))
    return dict(xt=xt, parts=parts)


def assemble_batch(d, nbh, nhalves):
    """Pack per-half sort data into device arrays for block budget nbh."""
    hp = HW // nhalves
    S = np.zeros((128, nhalves * nbh * 128), dtype=np.float16)
    blk_g = np.zeros(nhalves * nbh, dtype=np.int32)
    gsecs = []
    for h, (S_h, bg_h, pos_h, used_h) in enumerate(d['parts']):
        assert used_h <= nbh
        S[:, h * nbh * 128: h * nbh * 128 + nbh * 128] = S_h[:, :nbh * 128]
        blk_g[h * nbh: h * nbh + nbh] = bg_h[:nbh]
        for s in range(NUM_SAMPLES):
            gsecs.append(pos_h[:, s])
    # pre-gather slabs on the host: block b needs x^T rows [64g_b, 64g_b+128)
    # laid out partition-major so phase A is plain contiguous HWDGE streaming
    # (no SWDGE ring traffic at all in phase A)
    rows = 64 * blk_g[:, None] + np.arange(128)[None, :]      # [nblk, 128]
    xts = d["xt"][rows]                                       # [nblk, 128, CPAD]
    xts = np.ascontiguousarray(
        xts.transpose(1, 0, 2).reshape(128, -1))              # [128, nblk*CPAD]
    return {
        "xts": xts,
        "s_mat": S,
        "gat_idx": wrap16(np.concatenate(gsecs), HW * NUM_SAMPLES),
    }


def prep_weights(conv_w, conv_b):
    Wl = np.zeros((128, 3 * 9 * C), dtype=np.float16)
    for kc, (koff, ksz) in enumerate(MB):
        for tap in range(9):
            dy, dx = tap // 3 - 1, tap % 3 - 1
            for moff, msz in MB:
                blk = conv_w[moff:moff + msz, koff:koff + ksz, dy + 1, dx + 1]
                Wl[0:ksz, kc * 9 * C + tap * C + moff: kc * 9 * C + tap * C
                   + moff + msz] = blk.T.astype(np.float16)
    # paired kc=2 weights: rows 0:64 = tap (dy=0,dx), rows 64:128 = (dy=-1,dx)
    Wl2 = np.zeros((128, 3 * C), dtype=np.float16)
    for dxi, dx in enumerate((-1, 0, 1)):
        for moff, msz in MB:
            top = conv_w[moff:moff + msz, 256:320, 1, dx + 1]      # dy=0
            bot = conv_w[moff:moff + msz, 256:320, 0, dx + 1]      # dy=-1
            Wl2[0:64, dxi * C + moff: dxi * C + moff + msz] = \
                top.T.astype(np.float16)
            Wl2[64:128, dxi * C + moff: dxi * C + moff + msz] = \
                bot.T.astype(np.float16)
    bias = np.zeros((128, 3), dtype=np.float32)
    for mc, (moff, msz) in enumerate(MB):
        bias[0:msz, mc] = conv_b[moff:moff + msz].astype(np.float32)
    return Wl, Wl2, bias


# ------------------------------------------------------------- bass program

_NC_CACHE = {}


def build_program(reps=1, nblk=NBLK, nhalves=1):
    assert nblk % (A_CH * nhalves) == 0 and nblk <= NBLK
    key = (reps, nblk, nhalves)
    if key in _NC_CACHE:
        return _NC_CACHE[key]
    import concourse.bacc as bacc
    import concourse.mybir as mybir
    from concourse.tile import TileContext

    fp16 = mybir.dt.float16
    f32 = mybir.dt.float32
    i16 = mybir.dt.int16

    nc = bacc.Bacc(target_bir_lowering=False,
                   dynamic_dma_scratch_size=SCRATCH)
    xts_d = nc.dram_tensor("xts", [128, nblk * CPAD], fp16,
                           kind="ExternalInput")
    S = nc.dram_tensor("s_mat", [128, nblk * 128], fp16,
                       kind="ExternalInput")
    gidx_d = nc.dram_tensor("gat_idx", [128, 3 * HW // 16], i16,
                            kind="ExternalInput")
    wl_d = nc.dram_tensor("wl", [128, 3 * 9 * C], fp16, kind="ExternalInput")
    wl2_d = nc.dram_tensor("wl2", [128, 3 * C], fp16, kind="ExternalInput")
    bias_d = nc.dram_tensor("bias", [128, 3], f32, kind="ExternalInput")
    out_d = nc.dram_tensor("out", [C, HW], f32, kind="ExternalOutput")

    with TileContext(nc) as tc:
        with tc.tile_pool(name="const", bufs=1) as constp:
            wl = constp.tile([128, 3 * 9 * C], fp16)
            nc.sync.dma_start(out=wl[:], in_=wl_d[:])
            wl2 = constp.tile([128, 3 * C], fp16)
            nc.sync.dma_start(out=wl2[:], in_=wl2_d[:])
            bias_t = constp.tile([128, 3], f32)
            nc.sync.dma_start(out=bias_t[:], in_=bias_d[:])
            gidx = constp.tile([128, 3 * HW // 16], i16)
            nc.sync.dma_start(out=gidx[:], in_=gidx_d[:])

            def body(_it):
                with tc.tile_pool(name="val", bufs=1) as valp:
                    val = valp.tile([128, nblk * CPAD], fp16)
                    # zero the channel-pad region of every rank stripe (on
                    # DVE, keeping the Pool sequencer free for gather
                    # descriptor generation)
                    val3 = val.rearrange("p (b c) -> p b c", c=CPAD)
                    nc.vector.memset(val3[:, :, C:CPAD], 0.0)

                    # ---- phase A: sampling matmuls ----
                    with tc.tile_pool(name="slab", bufs=2) as slabp, \
                         tc.tile_pool(name="smat", bufs=2) as smatp, \
                         tc.tile_pool(name="psA", bufs=2, space="PSUM") as psA:
                        DR = 4   # blocks per drain group (4 psum banks)
                        for chk in range(nblk // A_CH):
                            nidx = A_CH * 128
                            slab = slabp.tile([128, A_CH * CPAD], fp16)
                            nc.sync.dma_start(
                                out=slab[:],
                                in_=xts_d[:, chk * A_CH * CPAD:
                                          (chk + 1) * A_CH * CPAD])
                            smat = smatp.tile([128, A_CH * 128], fp16)
                            nc.sync.dma_start(
                                out=smat[:],
                                in_=S[:, chk * nidx:(chk + 1) * nidx])
                            for g4 in range(A_CH // DR):
                                ps = psA.tile([128, DR, 512], f32)
                                for b4 in range(DR):
                                    b = g4 * DR + b4
                                    nc.tensor.matmul(
                                        ps[:, b4, 0:C],
                                        smat[:, b * 128:(b + 1) * 128],
                                        slab[:, b * CPAD:b * CPAD + C],
                                        start=True, stop=True)
                                blk0 = chk * A_CH + g4 * DR
                                if g4 % 3 != 2:
                                    nc.vector.tensor_copy(
                                        val3[:, blk0:blk0 + DR, 0:C],
                                        ps[:, :, 0:C])
                                else:
                                    nc.scalar.copy(
                                        val3[:, blk0:blk0 + DR, 0:C],
                                        ps[:, :, 0:C])

                    # ---- phase B: unsort + mean -> padded layout ----
                    with tc.tile_pool(name="samp", bufs=1) as sampp:
                        sampled = sampp.tile([128, 3 * PH * PW], fp16)
                        smp4 = sampled.rearrange("p (k r c) -> p k r c",
                                                 k=3, r=PH)
                        # zero only the pad borders (interior is overwritten)
                        nc.vector.memset(smp4[:, :, 0:1, :], 0.0)
                        nc.vector.memset(smp4[:, :, PH - 1:PH, :], 0.0)
                        nc.vector.memset(smp4[:, :, :, 0:1], 0.0)
                        nc.vector.memset(smp4[:, :, :, PW - 1:PW], 0.0)
                        # duplicated kc=2 plane for paired (dy=0,dy=-1) taps:
                        # partitions 0:64 hold D at offset 0, 64:128 at +PW
                        smp2d = sampp.tile([128, PH * PW + PW], fp16)
                        with tc.tile_pool(name="gout",
                                          bufs=(2 if nblk <= 144 else 1)) \
                                as goutp:
                            rows_per = B_CH // W
                            nbh = nblk // nhalves
                            hp = HW // nhalves
                            nqh = hp // B_CH
                            for h in range(nhalves):
                                # unsort gathers read only this half's val
                                # slice, so they can start (and run under
                                # phase A of the next half) as soon as this
                                # half's sampling drains land.
                                valh = val[:, h * nbh * CPAD:
                                           (h + 1) * nbh * CPAD]
                                for q in range(nqh):
                                    gs = []
                                    for s in range(NUM_SAMPLES):
                                        c0 = ((h * 3 + s) * hp
                                              + q * B_CH) // 16
                                        g = goutp.tile([128, 3, B_CH], fp16,
                                                       name=f"g{s}",
                                                       tag=f"g{s}")
                                        nc.gpsimd.dma_gather(
                                            out_ap=g[:],
                                            in_ap=valh,
                                            idxs_ap=gidx[:, c0:
                                                         c0 + B_CH // 16],
                                            num_idxs=B_CH,
                                            num_idxs_reg=B_CH,
                                            elem_size=CPAD,
                                            transpose=True,
                                            sbuf_tokens_per_rank=128,
                                            sbuf_free_dim_per_rank=CPAD * 2,
                                        )
                                        gs.append(g)
                                    tmp = goutp.tile([128, 3 * B_CH], fp16,
                                                     tag="tmp")
                                    nc.vector.tensor_add(
                                        tmp[:],
                                        gs[0].rearrange("p k n -> p (k n)"),
                                        gs[1].rearrange("p k n -> p (k n)"))
                                    qg = h * nqh + q
                                    r0 = 1 + qg * rows_per
                                    nc.vector.tensor_add(
                                        smp4[:, :, r0:r0 + rows_per, 1:1 + W],
                                        tmp.rearrange("p (k r c) -> p k r c",
                                                      k=3, c=W),
                                        gs[2].rearrange("p k (r c) -> p k r c",
                                                        c=W))
                                    # band-wise pair-plane copy
                                    nq = HW // B_CH
                                    lo = r0 if qg > 0 else 0
                                    hi = (r0 + rows_per if qg < nq - 1
                                          else PH)
                                    band = smp4[0:64, 2, lo:hi, :].rearrange(
                                        "p a b -> p (a b)")
                                    nc.sync.dma_start(
                                        out=smp2d[0:64, lo * PW:hi * PW],
                                        in_=band)
                                    nc.sync.dma_start(
                                        out=smp2d[64:128,
                                                  (lo + 1) * PW:(hi + 1) * PW],
                                        in_=band)

                        nc.vector.memset(smp2d[64:128, 0:PW], 0.0)

                        # ---- phase C: 3x3 conv + bias + relu ----
                        # 24 matmuls per (m, r): 18 full-K taps (kc 0/1),
                        # 3 paired kc=2 (dy 0/-1 stacked), 3 single (dy=+1).
                        with tc.tile_pool(name="psC", bufs=2, space="PSUM") \
                                as psC, \
                             tc.tile_pool(name="outp", bufs=4) as outp:
                            NCOL = 512
                            rows_n = NCOL // W
                            NR = HW // NCOL
                            N_MM = 24 if CONV_PAIR else 27
                            smp2f = smp2d  # [128, PH*PW + PW]
                            smp2v = smp2f.rearrange("p (r c) -> p r c",
                                                    c=PW)
                            # collect the 24 (lhsT, rhs) generators per
                            # m-chunk, then issue r-outer so the PE consumes
                            # sampled bands as phase B streams them
                            mqs = []
                            for mc, (moff, msz) in enumerate(MB):
                                mmq = []
                                kcs = (0, 1) if CONV_PAIR else (0, 1, 2)
                                for tap in range(9):
                                    dy, dx = tap // 3 - 1, tap % 3 - 1
                                    for kc in kcs:
                                        ksz = 128 if kc < 2 else 64
                                        lhsT = wl[0:ksz,
                                                  kc * 9 * C + tap * C + moff:
                                                  kc * 9 * C + tap * C + moff + msz]
                                        mmq.append((lhsT,
                                            lambda r, dy=dy, dx=dx, kc=kc,
                                            ksz=ksz:
                                            smp4[0:ksz, kc,
                                                 1 + dy + r * rows_n:
                                                 1 + dy + r * rows_n + rows_n,
                                                 1 + dx:1 + dx + W]))
                                if CONV_PAIR:
                                    for dxi, dx in enumerate((-1, 0, 1)):
                                        # paired dy=0 (p<64) + dy=-1 (p>=64)
                                        lhsT = wl2[0:128,
                                                   dxi * C + moff:
                                                   dxi * C + moff + msz]
                                        mmq.append((lhsT, lambda r, dx=dx:
                                            smp2v[0:128,
                                                  1 + r * rows_n:
                                                  1 + r * rows_n + rows_n,
                                                  1 + dx:1 + dx + W]))
                                        # single dy=+1, K=64
                                        lhsT1 = wl[0:64,
                                                   2 * 9 * C + (2 * 3 + dxi) * C + moff:
                                                   2 * 9 * C + (2 * 3 + dxi) * C
                                                   + moff + msz]
                                        mmq.append((lhsT1, lambda r, dx=dx:
                                            smp4[0:64, 2,
                                                 2 + r * rows_n:
                                                 2 + r * rows_n + rows_n,
                                                 1 + dx:1 + dx + W]))
                                assert len(mmq) == N_MM
                                mqs.append((moff, msz, mmq))
                            for r in range(NR):
                                for mc, (moff, msz, mmq) in enumerate(mqs):
                                    ps = psC.tile([128, NCOL], f32,
                                                  name=f"ps{mc}",
                                                  tag=f"ps{mc}")
                                    for j, (lhsT, rhs_fn) in enumerate(mmq):
                                        nc.tensor.matmul(
                                            ps[0:msz], lhsT, rhs_fn(r),
                                            start=(j == 0),
                                            stop=(j == N_MM - 1))
                                    ot = outp.tile([128, NCOL], f32)
                                    nc.scalar.activation(
                                        ot[0:msz], ps[0:msz],
                                        mybir.ActivationFunctionType.Relu,
                                        bias=bias_t[0:msz, mc:mc + 1])
                                    nc.sync.dma_start(
                                        out=out_d[moff:moff + msz,
                                                  r * NCOL:(r + 1) * NCOL],
                                        in_=ot[0:msz])

            if reps == 1:
                body(0)
            else:
                with tc.For_i(0, reps, 1) as it:
                    body(it)

    nc.finalize()
    _NC_CACHE[key] = nc
    return nc


# ---------------------------------------------------------------- interface

def make_in_maps(x, source_intrinsics, target_intrinsics, source_pose,
                 target_pose, conv_w, conv_b):
    F = fundamental_np(source_intrinsics, target_intrinsics,
                       source_pose, target_pose)
    Wl, Wl2, bias = prep_weights(conv_w, conv_b)
    nhalves = 2
    ds = [prep_batch(x[b], F[b], nhalves) for b in range(B)]
    nbh = max(p[3] for d in ds for p in d['parts'])
    nbh = ((nbh + A_CH - 1) // A_CH) * A_CH
    if nhalves * nbh > NBLK:
        # half-split padding too large for the SBUF budget; fall back to
        # one global sort
        nhalves = 1
        ds = [prep_batch(x[b], F[b], nhalves) for b in range(B)]
        nbh = max(p[3] for d in ds for p in d['parts'])
        nbh = min(NBLK, ((nbh + A_CH - 1) // A_CH) * A_CH)
    in_maps = []
    for b in range(B):
        m = assemble_batch(ds[b], nbh, nhalves)
        m.update({"wl": Wl, "wl2": Wl2, "bias": bias})
        in_maps.append(m)
    return in_maps, nhalves * nbh, nhalves


def kernel(x, source_intrinsics, target_intrinsics, source_pose,
           target_pose, conv_w, conv_b, _reps=1):
    from concourse.bass_utils import run_bass_kernel_spmd
    x = np.asarray(x, dtype=np.float32)
    in_maps = make_in_maps(
        x, np.asarray(source_intrinsics), np.asarray(target_intrinsics),
        np.asarray(source_pose), np.asarray(target_pose),
        np.asarray(conv_w, dtype=np.float32), np.asarray(conv_b, dtype=np.float32))
    in_maps, nblk, nhalves = in_maps
    nc = build_program(_reps, nblk, nhalves)
    res = run_bass_kernel_spmd(nc, in_maps, list(range(8)))
    out = np.stack([res.results[i]["out"].reshape(C, H, W) for i in range(8)])
    return out.astype(np.float32)



# revision 18
# speedup vs baseline: 4.5897x; 4.5897x over previous
"""Trainium2 Bass kernel for nn_EpipolarWarpOperator (B=8, C=320, H=W=64, S=3).

Sharding: spatial — every core computes an 8-row horizontal strip of all 8
batch outputs (the work of a batch is dominated by the 3x3 conv, which is
strip-separable; the epipolar sampling is handled per strip with halo).

Host analysis classifies each batch by its epipolar sampling map:

* pattern batches: the per-pixel bilinear sampling signature map has few
  distinct values (the epipolar lines clip to a handful of source locations),
  so the sampled image is piecewise constant and each output column is one of
  npat distinct "pattern" columns (3x3 signature neighborhoods incl. the
  zero-pad border). Per batch the device computes R[k, tap, pi] (replicated
  sampled columns) from a host-gathered slab via tiny matmuls,
  T^T[pi, m] = sum_{tap,kc} R_tap.T @ W_tap (+bias via a ones-row), relu on
  ACT, then expands out = T^T.T @ E with a per-core 0/1 selection matrix E.
  All x-dependent math stays on device.

* dense batches: per (128-pixel block, sample) the host pre-gathers the
  <=128 distinct bilinear corner pixels into a slab [128, 320] plus a
  sampling matrix S [128, 128]; the device runs swapped-operand matmuls
  slab.T @ S accumulating all samples (and sub-slabs) into PSUM, directly in
  channel-major order, then a 27-matmul 3x3 conv over the strip (+halo row)
  with bias+relu on ACT.
"""

import numpy as np

B, C, H, W = 8, 320, 64, 64
SN = 3
HW = H * W
STRIP = 8             # output rows per core
NCORE = 8
MB = [(0, 128), (128, 128), (256, 64)]   # output/input channel chunking
PI_MAX = 126          # total pattern budget (K of the expansion matmul)
NPAT_MAX = 56         # per-batch pattern cap (9*npat*4B must fit a PSUM bank)
NSIG_MAX = 64
NBLK = 6              # 2-row sampling blocks per strip (incl. 2-row halo pad)

# ---------------------------------------------------------------- host: geometry


def _rodrigues_np(rv):
    theta = np.sqrt((rv * rv).sum())
    r = rv / max(theta, 1e-12)
    I = np.eye(3, dtype=np.float32)
    K = np.array([[0, -r[2], r[1]], [r[2], 0, -r[0]], [-r[1], r[0], 0]],
                 dtype=np.float32)
    R = np.cos(theta) * I + (1 - np.cos(theta)) * np.outer(r, r) + np.sin(theta) * K
    return I if theta < 1e-6 else R


def fundamental_np(Ks, Kt, ps, pt):
    Fs = []
    for b in range(Ks.shape[0]):
        Rs = _rodrigues_np(ps[b, :3].astype(np.float32))
        Rt = _rodrigues_np(pt[b, :3].astype(np.float32))
        ts_, tt_ = ps[b, 3:].astype(np.float32), pt[b, 3:].astype(np.float32)
        R_rel = Rs @ Rt.T
        t_rel = ts_ - R_rel @ tt_
        z = np.float32(0)
        skew = np.array([[z, -t_rel[2], t_rel[1]],
                         [t_rel[2], z, -t_rel[0]],
                         [-t_rel[1], t_rel[0], z]], dtype=np.float32)
        E = skew @ R_rel
        inv_Ks = np.linalg.inv(Ks[b].astype(np.float32))
        inv_Kt = np.linalg.inv(Kt[b].astype(np.float32))
        Fs.append(inv_Kt.T @ E @ inv_Ks)
    return np.stack(Fs).astype(np.float32)


def geometry(F):
    k = np.arange(HW)
    px = (k % W).astype(np.float32)
    py = (k // W).astype(np.float32)
    P = np.stack([px, py, np.ones_like(px)])
    lines = F.T.astype(np.float32) @ P
    a, b_, c = lines[0], lines[1], lines[2]
    W1, H1 = np.float32(W - 1), np.float32(H - 1)
    EPS = np.float32(1e-10)
    x1 = np.clip(-c / (a + EPS), 0.0, W1)
    x2 = np.clip(-(b_ * H1 + c) / (a + EPS), 0.0, W1)
    y1 = np.clip(-c / (b_ + EPS), 0.0, H1)
    y2 = np.clip(-(a * W1 + c) / (b_ + EPS), 0.0, H1)
    t = np.linspace(0.0, 1.0, SN, dtype=np.float32)
    sx = x1[:, None] * (1 - t) + x2[:, None] * t
    sy = y1[:, None] * (1 - t) + y2[:, None] * t
    x0 = np.floor(sx)
    y0 = np.floor(sy)
    wx = (sx - x0).astype(np.float32)
    wy = (sy - y0).astype(np.float32)
    x0i = np.clip(x0, 0, W - 1).astype(np.int32)
    y0i = np.clip(y0, 0, H - 1).astype(np.int32)
    return x0i, y0i, wx, wy


def _corners(geo, p, s):
    """4 bilinear (source pixel row, weight/3) pairs for pixel p, sample s."""
    x0i, y0i, wx, wy = geo
    y0 = int(y0i[p, s]); x0 = int(x0i[p, s])
    x1 = min(x0 + 1, W - 1); y1 = min(y0 + 1, H - 1)
    wxx = np.float32(wx[p, s]); wyy = np.float32(wy[p, s])
    third = np.float32(1.0 / 3.0)
    out = {}
    for ry, rx, ww in ((y0, x0, (1 - wxx) * (1 - wyy)),
                       (y0, x1, wxx * (1 - wyy)),
                       (y1, x0, (1 - wxx) * wyy),
                       (y1, x1, wxx * wyy)):
        rr = ry * W + rx
        out[rr] = out.get(rr, np.float32(0)) + ww * third
    return out


# ------------------------------------------------------------- host: classify


def classify(geo):
    """If the sampling map has <= NSIG_MAX distinct per-pixel signatures,
    return (siginv [HW], tables); else None."""
    x0i, y0i, wx, wy = geo
    key = np.concatenate([
        y0i.astype(np.float32), x0i.astype(np.float32), wx, wy], axis=1)
    kview = np.ascontiguousarray(key).view([('', key.dtype)] * key.shape[1]).ravel()
    uniq, first, inv = np.unique(kview, return_index=True, return_inverse=True)
    if len(uniq) > NSIG_MAX:
        return None
    tables = []
    for si in range(len(uniq)):
        p = int(first[si])
        contrib = {}
        for s in range(SN):
            for rr, ww in _corners(geo, p, s).items():
                contrib[rr] = contrib.get(rr, np.float32(0)) + ww
        tables.append(sorted(contrib.items()))
    return inv.astype(np.int32), tables


def patterns(siginv):
    """3x3 neighborhood patterns of the signature map (border = -1)."""
    simg = siginv.reshape(H, W)
    pad = np.pad(simg, 1, constant_values=-1)
    neigh = np.stack([pad[dy:dy + H, dx:dx + W].ravel()
                      for dy in range(3) for dx in range(3)], axis=1)
    uniq, pinv = np.unique(neigh, axis=0, return_inverse=True)
    return pinv.astype(np.int32), uniq


# ------------------------------------------------------------ host: build plan


def _deg_slabs(tables):
    """Assign signatures to slabs so each slab's source-pixel union <= 128.
    Returns list of (rowmap, sigs) with rowmap = {pixrow: slabrow}."""
    slabs = []
    rows, sigs = {}, []
    for si, tab in enumerate(tables):
        new = [rr for rr, _ in tab if rr not in rows]
        if len(rows) + len(new) > 128:
            slabs.append((rows, sigs))
            rows, sigs = {}, []
            new = [rr for rr, _ in tab]
        for rr in new:
            rows[rr] = len(rows)
        sigs.append(si)
    slabs.append((rows, sigs))
    return slabs


def _dense_block_slabs(pix_ids, geo):
    """Greedy split of a 128-pixel block's (pixel, sample) tokens into slabs
    of <=128 distinct corner pixels, tokens sorted by corner location so
    nearby tokens share slab rows. Returns list of (rowmap, S[128,128])."""
    x0i, y0i = geo[0], geo[1]
    toks = [(int(y0i[p, s]), int(x0i[p, s]), i, p, s)
            for i, p in enumerate(pix_ids) if p >= 0 for s in range(SN)]
    toks.sort()
    slabs = []
    rows = {}
    S = np.zeros((128, 128), dtype=np.float32)
    for _, _, i, p, s in toks:
        cs = _corners(geo, p, s)
        new = [rr for rr in cs if rr not in rows]
        if len(rows) + len(new) > 128:
            slabs.append((rows, S))
            rows = {}
            S = np.zeros((128, 128), dtype=np.float32)
            new = list(cs)
        for rr in new:
            rows[rr] = len(rows)
        for rr, ww in cs.items():
            S[rows[rr], i] += ww
    if rows or not slabs:
        slabs.append((rows, S))
    return slabs


def make_plan(x, source_intrinsics, target_intrinsics, source_pose,
              target_pose, conv_w, conv_b):
    Fs = fundamental_np(np.asarray(source_intrinsics, np.float32),
                        np.asarray(target_intrinsics, np.float32),
                        np.asarray(source_pose, np.float32),
                        np.asarray(target_pose, np.float32))
    x = np.asarray(x, np.float32)
    xT16 = [np.ascontiguousarray(x[b].reshape(C, HW).T).astype(np.float16)
            for b in range(B)]

    degs, denses = [], []
    pi_used = 0
    for b in range(B):
        geo = geometry(Fs[b])
        res = classify(geo)
        if res is not None:
            siginv, tables = res
            pinv, ptab = patterns(siginv)
            npat = ptab.shape[0]
            if npat <= NPAT_MAX and pi_used + npat <= PI_MAX:
                degs.append(dict(gb=b, siginv=siginv, tables=tables,
                                 pinv=pinv, ptab=ptab, pi_off=pi_used))
                pi_used += npat
                continue
        denses.append(dict(gb=b, geo=geo))
    pi_tot = pi_used

    # ---- degenerate global data (same for all cores) ----
    sg_blocks, sr_blocks = [], []
    for d in degs:
        tables, ptab = d['tables'], d['ptab']
        npat = ptab.shape[0]
        slabs = _deg_slabs(tables)
        d['nslab'] = len(slabs)
        for rows, sigs in slabs:
            slab = np.zeros((128, C), dtype=np.float16)
            rl = sorted(rows, key=rows.get)
            if rl:
                slab[:len(rl)] = xT16[d['gb']][np.array(rl)]
            sigset = set(sigs)
            SR = np.zeros((128, 9 * npat), dtype=np.float32)
            for pi in range(npat):
                for tap in range(9):
                    si = ptab[pi, tap]
                    if si >= 0 and si in sigset:
                        for rr, ww in tables[si]:
                            SR[rows[rr], tap * npat + pi] += ww
            sg_blocks.append(slab)
            sr_blocks.append(SR.astype(np.float16))
    sg = (np.concatenate(sg_blocks, axis=1) if sg_blocks
          else np.zeros((128, 0), np.float16))
    sr = (np.concatenate(sr_blocks, axis=1) if sr_blocks
          else np.zeros((128, 0), np.float16))

    # ---- dense per-core blocks ----
    # block = 2 image rows 8r-2+2*blk, +1 of a dense batch (halo included;
    # out-of-image rows get zero columns); all 3 samples share its slabs
    core_units = []   # [core][block] -> list of (rowmap, S)
    for r in range(NCORE):
        units = []
        for d in denses:
            for blk in range(NBLK):
                row0 = 8 * r - 2 + 2 * blk
                pix = []
                for lr in range(2):
                    row = row0 + lr
                    pix += [row * W + cx if 0 <= row < H else -1
                            for cx in range(W)]
                units.append(_dense_block_slabs(pix, d['geo']))
        core_units.append(units)
    nunits = len(core_units[0])
    unit_nslab = [max(1, max(len(core_units[r][u]) for r in range(NCORE)))
                  for u in range(nunits)]

    ui = 0
    for d in denses:
        d['unit_nslab'] = tuple(unit_nslab[ui:ui + NBLK])
        ui += NBLK

    # ---- weights ----
    Wl = np.zeros((128, 3 * 9 * C), dtype=np.float16)
    cw = np.asarray(conv_w, np.float32)
    cb = np.asarray(conv_b, np.float32)
    for kc, (koff, ksz) in enumerate(MB):
        for tap in range(9):
            dy, dx = tap // 3, tap % 3
            Wl[0:ksz, kc * 9 * C + tap * C: kc * 9 * C + tap * C + C] = \
                cw[:, koff:koff + ksz, dy, dx].T.astype(np.float16)
    bias = np.zeros((128, 3), dtype=np.float32)
    for mc, (moff, msz) in enumerate(MB):
        bias[0:msz, mc] = cb[moff:moff + msz]

    # ---- per-core in_maps ----
    ndeg, ndense = len(degs), len(denses)
    slots = [d['gb'] for d in degs] + [d['gb'] for d in denses]
    in_maps = []
    for r in range(NCORE):
        m = {"wl": Wl, "bias": bias}
        if ndeg:
            m["sg"] = sg
            m["sr"] = sr
            e = np.zeros((128, ndeg * STRIP * W), dtype=np.float16)
            for bi, d in enumerate(degs):
                pidx = d['pinv'].reshape(H, W)[8 * r: 8 * r + STRIP].ravel()
                e[d['pi_off'] + pidx,
                  bi * STRIP * W + np.arange(STRIP * W)] = 1.0
            m["e_mat"] = e
        if ndense:
            sds, sss = [], []
            for u in range(nunits):
                slabs = core_units[r][u]
                di = u // NBLK
                gb = denses[di]['gb']
                for j in range(unit_nslab[u]):
                    slab = np.zeros((128, C), dtype=np.float16)
                    S = np.zeros((128, 128), dtype=np.float16)
                    if j < len(slabs):
                        rows, Sf = slabs[j]
                        rl = sorted(rows, key=rows.get)
                        if rl:
                            slab[:len(rl)] = xT16[gb][np.array(rl)]
                        S = Sf.astype(np.float16)
                    sds.append(slab)
                    sss.append(S)
            m["sd"] = np.concatenate(sds, axis=1)
            m["ss"] = np.concatenate(sss, axis=1)
        in_maps.append(m)

    struct = (pi_tot,
              tuple((d['gb'], d['ptab'].shape[0], d['nslab']) for d in degs),
              tuple((d['gb'], d['unit_nslab']) for d in denses))
    return in_maps, struct, slots


# ------------------------------------------------------------- bass program

_NC_CACHE = {}


def build_program(reps, struct):
    key = (reps, struct)
    if key in _NC_CACHE:
        return _NC_CACHE[key]
    import concourse.bacc as bacc
    import concourse.mybir as mybir
    from concourse.tile import TileContext

    fp16 = mybir.dt.float16
    f32 = mybir.dt.float32
    pi_tot, degs, denses = struct
    ndeg, ndense = len(degs), len(denses)
    NB = ndeg + ndense
    NSG = sum(ns for _, _, ns in degs)
    SRC = sum(9 * npat * ns for _, npat, ns in degs)
    NSLAB = sum(sum(us) for _, us in denses)
    SW = STRIP * W   # 512 pixels per strip

    nc = bacc.Bacc(target_bir_lowering=False)
    wl_d = nc.dram_tensor("wl", [128, 3 * 9 * C], fp16, kind="ExternalInput")
    bias_d = nc.dram_tensor("bias", [128, 3], f32, kind="ExternalInput")
    if ndeg:
        sg_d = nc.dram_tensor("sg", [128, NSG * C], fp16, kind="ExternalInput")
        sr_d = nc.dram_tensor("sr", [128, SRC], fp16, kind="ExternalInput")
        e_d = nc.dram_tensor("e_mat", [128, ndeg * SW], fp16,
                             kind="ExternalInput")
    if ndense:
        sd_d = nc.dram_tensor("sd", [128, NSLAB * C], fp16,
                              kind="ExternalInput")
        ss_d = nc.dram_tensor("ss", [128, NSLAB * 128], fp16,
                              kind="ExternalInput")
    out_d = nc.dram_tensor("out", [128, NB * 3 * SW], fp16,
                           kind="ExternalOutput")

    with TileContext(nc) as tc:
        with tc.tile_pool(name="const", bufs=1) as constp:
            wl = constp.tile([128, 3 * 9 * C], fp16)
            nc.sync.dma_start(out=wl[:], in_=wl_d[:])
            bias_t = constp.tile([128, 3], f32)
            nc.sync.dma_start(out=bias_t[:], in_=bias_d[:])

            def body(_it):
                with tc.tile_pool(name="inp", bufs=2) as inp, \
                     tc.tile_pool(name="sdp", bufs=1) as sdp, \
                     tc.tile_pool(name="ssp", bufs=1) as ssp, \
                     tc.tile_pool(name="work", bufs=1) as work, \
                     tc.tile_pool(name="smpp", bufs=2) as smpp, \
                     tc.tile_pool(name="outp", bufs=2) as outp, \
                     tc.tile_pool(name="psA", bufs=3, space="PSUM") as psA, \
                     tc.tile_pool(name="psT", bufs=1, space="PSUM") as psT, \
                     tc.tile_pool(name="psB", bufs=3, space="PSUM") as psB:

                    out_sb = outp.tile([128, NB, 3, SW], fp16, name="out_sb")
                    # mc=2 has only 64 valid channel partitions; zero the rest
                    # so the out DMA never reads uninitialized SBUF
                    nc.vector.memset(out_sb[64:128, :, 2:3, :], 0.0)

                    # ---------- input DMAs ----------
                    if ndeg:
                        sg = inp.tile([128, NSG * C], fp16, name="sg",
                                      tag="sg")
                        nc.sync.dma_start(out=sg[:], in_=sg_d[:])
                        sr = inp.tile([128, SRC], fp16, name="sr", tag="sr")
                        nc.sync.dma_start(out=sr[:], in_=sr_d[:])
                        e = inp.tile([128, ndeg * SW], fp16, name="e",
                                     tag="e")
                        nc.sync.dma_start(out=e[:], in_=e_d[:])
                    sd_tiles, ss_tiles = [], []
                    if ndense:
                        # chunk slab streams per (dense batch, block)
                        off = 0
                        for di, (_, us) in enumerate(denses):
                            for blk in range(NBLK):
                                nsl = us[blk]
                                sdt = sdp.tile([128, nsl * C], fp16,
                                               tag=f"sd{blk}")
                                nc.sync.dma_start(
                                    out=sdt[:],
                                    in_=sd_d[:, off * C:(off + nsl) * C])
                                sst = ssp.tile([128, nsl * 128], fp16,
                                               tag=f"ss{blk}")
                                nc.sync.dma_start(
                                    out=sst[:],
                                    in_=ss_d[:, off * 128:(off + nsl) * 128])
                                sd_tiles.append(sdt)
                                ss_tiles.append(sst)
                                off += nsl

                    # ---------- degenerate path: R ----------
                    if ndeg:
                        rsb = work.tile([128, 3, 9, pi_tot], fp16, name="rsb")
                        sgo, sro, poff = 0, 0, 0
                        for bi, (_, npat, nsl) in enumerate(degs):
                            ps_r = psA.tile([128, 512], f32, name=f"psr{bi}",
                                            tag="psA")
                            for kc, (koff, ksz) in enumerate(MB):
                                for j in range(nsl):
                                    nc.tensor.matmul(
                                        ps_r[0:ksz, kc * 9 * npat:
                                             (kc + 1) * 9 * npat],
                                        sg[:, sgo + j * C + koff:
                                           sgo + j * C + koff + ksz],
                                        sr[:, sro + j * 9 * npat:
                                           sro + (j + 1) * 9 * npat],
                                        start=(j == 0), stop=(j == nsl - 1))
                            psv = ps_r[:, 0:27 * npat].rearrange(
                                "p (k t q) -> p k t q", k=3, t=9)
                            nc.vector.tensor_copy(
                                rsb[0:128, 0:2, :, poff:poff + npat],
                                psv[0:128, 0:2, :, :])
                            nc.vector.tensor_copy(
                                rsb[0:64, 2:3, :, poff:poff + npat],
                                psv[0:64, 2:3, :, :])
                            sgo += nsl * C
                            sro += nsl * 9 * npat
                            poff += npat

                    # ---------- dense sampling (first half) ----------
                    smps = []
                    if ndense:
                        for di in range(ndense):
                            smp = smpp.tile([128, 3, 2 * NBLK, 66], fp16,
                                            name=f"smp{di}", tag=f"smp{di}")
                            nc.vector.memset(smp[:, :, :, 0:1], 0.0)
                            nc.vector.memset(smp[:, :, :, 65:66], 0.0)
                            smps.append(smp)

                    def dense_block(di, blk):
                        _, us = denses[di]
                        smp = smps[di]
                        sdt = sd_tiles[di * NBLK + blk]
                        sst = ss_tiles[di * NBLK + blk]
                        ps = psA.tile([128, 512], f32,
                                      name=f"psb{di}_{blk}", tag="psA")
                        nsl = us[blk]
                        for kc, (koff, ksz) in enumerate(MB):
                            for k in range(nsl):
                                nc.tensor.matmul(
                                    ps[0:ksz, kc * 128:(kc + 1) * 128],
                                    sdt[:, k * C + koff:
                                        k * C + koff + ksz],
                                    sst[:, k * 128:(k + 1) * 128],
                                    start=(k == 0), stop=(k == nsl - 1))
                        psv = ps[:, 0:384].rearrange("p (k r c) -> p k r c",
                                                     k=3, r=2)
                        nc.vector.tensor_copy(
                            smp[0:128, 0:2, 2 * blk:2 * blk + 2, 1:65],
                            psv[0:128, 0:2, :, :])
                        nc.vector.tensor_copy(
                            smp[0:64, 2:3, 2 * blk:2 * blk + 2, 1:65],
                            psv[0:64, 2:3, :, :])

                    if ndense:
                        for di in range(ndense):
                            for blk in range(3):
                                dense_block(di, blk)

                    # ---------- degenerate path: T ----------
                    # bias + relu commute with the per-pixel column selection,
                    # so they are applied after the expansion matmul instead
                    if ndeg:
                        ps_t = psT.tile([128, 512], f32, name="ps_t")
                        k = 0
                        for kc, (koff, ksz) in enumerate(MB):
                            for tap in range(9):
                                nc.tensor.matmul(
                                    ps_t[0:pi_tot, 0:C],
                                    rsb[0:ksz, kc, tap, :],
                                    wl[0:ksz, kc * 9 * C + tap * C:
                                       kc * 9 * C + tap * C + C],
                                    start=(k == 0), stop=(k == 26))
                                k += 1
                        tsb = work.tile([128, C], fp16, name="tsb")
                        nc.scalar.copy(tsb[0:pi_tot, :], ps_t[0:pi_tot, 0:C])

                    # ---------- dense sampling (second half) ----------
                    if ndense:
                        for di in range(ndense):
                            for blk in range(3, NBLK):
                                dense_block(di, blk)

                    # ---------- degenerate path: expansion ----------
                    if ndeg:
                        for mc, (moff, msz) in enumerate(MB):
                            for bi in range(ndeg):
                                ps_e = psB.tile([128, 512], f32,
                                                name=f"pse{mc}_{bi}",
                                                tag="psB")
                                nc.tensor.matmul(
                                    ps_e[0:msz, :],
                                    tsb[0:pi_tot, moff:moff + msz],
                                    e[0:pi_tot, bi * SW:(bi + 1) * SW],
                                    start=True, stop=True)
                                dst = out_sb[0:msz, bi, mc, :]
                                if (mc + bi) % 2 == 0:
                                    nc.vector.tensor_scalar(
                                        dst, ps_e[0:msz, :],
                                        bias_t[0:msz, mc:mc + 1], 0.0,
                                        mybir.AluOpType.add,
                                        mybir.AluOpType.max)
                                else:
                                    nc.scalar.activation(
                                        dst, ps_e[0:msz, :],
                                        mybir.ActivationFunctionType.Relu,
                                        bias=bias_t[0:msz, mc:mc + 1])
                        for bi in range(ndeg):
                            nc.sync.dma_start(
                                out=out_d[:, bi * 3 * SW:(bi + 1) * 3 * SW],
                                in_=out_sb[:, bi, :, :])

                    # ---------- dense conv + bias + relu ----------
                    if ndense:
                        for di in range(ndense):
                            smp = smps[di]
                            slot = ndeg + di
                            for mc, (moff, msz) in enumerate(MB):
                                ps_c = psB.tile([128, 512], f32,
                                                name=f"psc{di}_{mc}",
                                                tag="psB")
                                k = 0
                                for kc, (koff, ksz) in enumerate(MB):
                                    for tap in range(9):
                                        dy, dx = tap // 3, tap % 3
                                        nc.tensor.matmul(
                                            ps_c[0:msz, :],
                                            wl[0:ksz,
                                               kc * 9 * C + tap * C + moff:
                                               kc * 9 * C + tap * C + moff + msz],
                                            smp[0:ksz, kc, 1 + dy:9 + dy,
                                                dx:dx + 64],
                                            start=(k == 0), stop=(k == 26))
                                        k += 1
                                nc.scalar.activation(
                                    out_sb[0:msz, slot, mc, :], ps_c[0:msz, :],
                                    mybir.ActivationFunctionType.Relu,
                                    bias=bias_t[0:msz, mc:mc + 1])
                            nc.sync.dma_start(
                                out=out_d[:, slot * 3 * SW:
                                          (slot + 1) * 3 * SW],
                                in_=out_sb[:, slot, :, :])

            if reps == 1:
                body(0)
            else:
                with tc.For_i(0, reps, 1) as it:
                    body(it)

    nc.finalize()
    _NC_CACHE[key] = nc
    return nc


# ---------------------------------------------------------------- interface


def make_in_maps(x, source_intrinsics, target_intrinsics, source_pose,
                 target_pose, conv_w, conv_b):
    return make_plan(x, source_intrinsics, target_intrinsics, source_pose,
                     target_pose, conv_w, conv_b)


def assemble(results, slots):
    """results: list of per-core {"out": [128, NB*3*SW]} -> [B, C, H, W]."""
    out = np.zeros((B, C, H, W), dtype=np.float32)
    NBl = len(slots)
    for r in range(NCORE):
        o = np.asarray(results[r]["out"]).reshape(128, NBl, 3, STRIP, W)
        for si, gb in enumerate(slots):
            for mc, (moff, msz) in enumerate(MB):
                out[gb, moff:moff + msz, 8 * r: 8 * r + STRIP, :] = \
                    o[0:msz, si, mc].astype(np.float32)
    return out


def kernel(x, source_intrinsics, target_intrinsics, source_pose,
           target_pose, conv_w, conv_b, _reps=1):
    from concourse.bass_utils import run_bass_kernel_spmd
    in_maps, struct, slots = make_in_maps(
        x, source_intrinsics, target_intrinsics, source_pose,
        target_pose, conv_w, conv_b)
    nc = build_program(_reps, struct)
    res = run_bass_kernel_spmd(nc, in_maps, list(range(NCORE)))
    return assemble(res.results, slots)


# revision 22
# speedup vs baseline: 6.0893x; 1.3268x over previous
"""Trainium2 Bass kernel for nn_EpipolarWarpOperator (B=8, C=320, H=W=64, S=3).

Sharding: spatial — every core computes an 8-row horizontal strip of all 8
batch outputs (the work of a batch is dominated by the 3x3 conv, which is
strip-separable; the epipolar sampling is handled per strip with halo).

Host analysis classifies each batch by its epipolar sampling map:

* pattern batches: the per-pixel bilinear sampling signature map has few
  distinct values (the epipolar lines clip to a handful of source locations),
  so the sampled image is piecewise constant and each output column is one of
  npat distinct "pattern" columns (3x3 signature neighborhoods incl. the
  zero-pad border). Per batch the device computes R[k, tap, pi] (replicated
  sampled columns) from a host-gathered slab via tiny matmuls,
  T^T[pi, m] = sum_{tap,kc} R_tap.T @ W_tap (+bias via a ones-row), relu on
  ACT, then expands out = T^T.T @ E with a per-core 0/1 selection matrix E.
  All x-dependent math stays on device.

* dense batches: per (128-pixel block, sample) the host pre-gathers the
  <=128 distinct bilinear corner pixels into a slab [128, 320] plus a
  sampling matrix S [128, 128]; the device runs swapped-operand matmuls
  slab.T @ S accumulating all samples (and sub-slabs) into PSUM, directly in
  channel-major order, then a 27-matmul 3x3 conv over the strip (+halo row)
  with bias+relu on ACT.
"""

import numpy as np

B, C, H, W = 8, 320, 64, 64
SN = 3
HW = H * W
STRIP = 8             # output rows per core
NCORE = 8
MB = [(0, 128), (128, 128), (256, 64)]   # output/input channel chunking
PI_MAX = 126          # total pattern budget (K of the expansion matmul)
NPAT_MAX = 56         # per-batch pattern cap (9*npat*4B must fit a PSUM bank)
NSIG_MAX = 64
NBLK = 6              # 2-row sampling blocks per strip (incl. 2-row halo pad)

# ---------------------------------------------------------------- host: geometry


def _rodrigues_np(rv):
    theta = np.sqrt((rv * rv).sum())
    r = rv / max(theta, 1e-12)
    I = np.eye(3, dtype=np.float32)
    K = np.array([[0, -r[2], r[1]], [r[2], 0, -r[0]], [-r[1], r[0], 0]],
                 dtype=np.float32)
    R = np.cos(theta) * I + (1 - np.cos(theta)) * np.outer(r, r) + np.sin(theta) * K
    return I if theta < 1e-6 else R


def fundamental_np(Ks, Kt, ps, pt):
    Fs = []
    for b in range(Ks.shape[0]):
        Rs = _rodrigues_np(ps[b, :3].astype(np.float32))
        Rt = _rodrigues_np(pt[b, :3].astype(np.float32))
        ts_, tt_ = ps[b, 3:].astype(np.float32), pt[b, 3:].astype(np.float32)
        R_rel = Rs @ Rt.T
        t_rel = ts_ - R_rel @ tt_
        z = np.float32(0)
        skew = np.array([[z, -t_rel[2], t_rel[1]],
                         [t_rel[2], z, -t_rel[0]],
                         [-t_rel[1], t_rel[0], z]], dtype=np.float32)
        E = skew @ R_rel
        inv_Ks = np.linalg.inv(Ks[b].astype(np.float32))
        inv_Kt = np.linalg.inv(Kt[b].astype(np.float32))
        Fs.append(inv_Kt.T @ E @ inv_Ks)
    return np.stack(Fs).astype(np.float32)


def geometry(F):
    k = np.arange(HW)
    px = (k % W).astype(np.float32)
    py = (k // W).astype(np.float32)
    P = np.stack([px, py, np.ones_like(px)])
    lines = F.T.astype(np.float32) @ P
    a, b_, c = lines[0], lines[1], lines[2]
    W1, H1 = np.float32(W - 1), np.float32(H - 1)
    EPS = np.float32(1e-10)
    x1 = np.clip(-c / (a + EPS), 0.0, W1)
    x2 = np.clip(-(b_ * H1 + c) / (a + EPS), 0.0, W1)
    y1 = np.clip(-c / (b_ + EPS), 0.0, H1)
    y2 = np.clip(-(a * W1 + c) / (b_ + EPS), 0.0, H1)
    t = np.linspace(0.0, 1.0, SN, dtype=np.float32)
    sx = x1[:, None] * (1 - t) + x2[:, None] * t
    sy = y1[:, None] * (1 - t) + y2[:, None] * t
    x0 = np.floor(sx)
    y0 = np.floor(sy)
    wx = (sx - x0).astype(np.float32)
    wy = (sy - y0).astype(np.float32)
    x0i = np.clip(x0, 0, W - 1).astype(np.int32)
    y0i = np.clip(y0, 0, H - 1).astype(np.int32)
    return x0i, y0i, wx, wy


def _corners(geo, p, s):
    """4 bilinear (source pixel row, weight/3) pairs for pixel p, sample s."""
    x0i, y0i, wx, wy = geo
    y0 = int(y0i[p, s]); x0 = int(x0i[p, s])
    x1 = min(x0 + 1, W - 1); y1 = min(y0 + 1, H - 1)
    wxx = np.float32(wx[p, s]); wyy = np.float32(wy[p, s])
    third = np.float32(1.0 / 3.0)
    out = {}
    for ry, rx, ww in ((y0, x0, (1 - wxx) * (1 - wyy)),
                       (y0, x1, wxx * (1 - wyy)),
                       (y1, x0, (1 - wxx) * wyy),
                       (y1, x1, wxx * wyy)):
        rr = ry * W + rx
        out[rr] = out.get(rr, np.float32(0)) + ww * third
    return out


# ------------------------------------------------------------- host: classify


def classify(geo):
    """If the sampling map has <= NSIG_MAX distinct per-pixel signatures,
    return (siginv [HW], tables); else None."""
    x0i, y0i, wx, wy = geo
    key = np.concatenate([
        y0i.astype(np.float32), x0i.astype(np.float32), wx, wy], axis=1)
    kview = np.ascontiguousarray(key).view([('', key.dtype)] * key.shape[1]).ravel()
    uniq, first, inv = np.unique(kview, return_index=True, return_inverse=True)
    if len(uniq) > NSIG_MAX:
        return None
    tables = []
    for si in range(len(uniq)):
        p = int(first[si])
        contrib = {}
        for s in range(SN):
            for rr, ww in _corners(geo, p, s).items():
                contrib[rr] = contrib.get(rr, np.float32(0)) + ww
        tables.append(sorted(contrib.items()))
    return inv.astype(np.int32), tables


def patterns(siginv):
    """3x3 neighborhood patterns of the signature map (border = -1)."""
    simg = siginv.reshape(H, W)
    pad = np.pad(simg, 1, constant_values=-1)
    neigh = np.stack([pad[dy:dy + H, dx:dx + W].ravel()
                      for dy in range(3) for dx in range(3)], axis=1)
    uniq, pinv = np.unique(neigh, axis=0, return_inverse=True)
    return pinv.astype(np.int32), uniq


# ------------------------------------------------------------ host: build plan


def _deg_slabs(tables):
    """Assign signatures to slabs so each slab's source-pixel union <= 128.
    Returns list of (rowmap, sigs) with rowmap = {pixrow: slabrow}."""
    slabs = []
    rows, sigs = {}, []
    for si, tab in enumerate(tables):
        new = [rr for rr, _ in tab if rr not in rows]
        if len(rows) + len(new) > 128:
            slabs.append((rows, sigs))
            rows, sigs = {}, []
            new = [rr for rr, _ in tab]
        for rr in new:
            rows[rr] = len(rows)
        sigs.append(si)
    slabs.append((rows, sigs))
    return slabs


def _dense_block_slabs(pix_ids, geo):
    """Greedy split of a 128-pixel block's (pixel, sample) tokens into slabs
    of <=128 distinct corner pixels, tokens sorted by corner location so
    nearby tokens share slab rows. Returns list of (rowmap, S[128,128])."""
    x0i, y0i = geo[0], geo[1]
    toks = [(int(y0i[p, s]), int(x0i[p, s]), i, p, s)
            for i, p in enumerate(pix_ids) if p >= 0 for s in range(SN)]
    toks.sort()
    slabs = []
    rows = {}
    S = np.zeros((128, 128), dtype=np.float32)
    for _, _, i, p, s in toks:
        cs = _corners(geo, p, s)
        new = [rr for rr in cs if rr not in rows]
        if len(rows) + len(new) > 128:
            slabs.append((rows, S))
            rows = {}
            S = np.zeros((128, 128), dtype=np.float32)
            new = list(cs)
        for rr in new:
            rows[rr] = len(rows)
        for rr, ww in cs.items():
            S[rows[rr], i] += ww
    if rows or not slabs:
        slabs.append((rows, S))
    return slabs


def make_plan(x, source_intrinsics, target_intrinsics, source_pose,
              target_pose, conv_w, conv_b):
    Fs = fundamental_np(np.asarray(source_intrinsics, np.float32),
                        np.asarray(target_intrinsics, np.float32),
                        np.asarray(source_pose, np.float32),
                        np.asarray(target_pose, np.float32))
    x = np.asarray(x, np.float32)
    xT16 = [np.ascontiguousarray(x[b].reshape(C, HW).T).astype(np.float16)
            for b in range(B)]

    degs, denses = [], []
    pi_used = 0
    for b in range(B):
        geo = geometry(Fs[b])
        res = classify(geo)
        if res is not None:
            siginv, tables = res
            pinv, ptab = patterns(siginv)
            npat = ptab.shape[0]
            if npat <= NPAT_MAX and pi_used + npat <= PI_MAX:
                degs.append(dict(gb=b, siginv=siginv, tables=tables,
                                 pinv=pinv, ptab=ptab, pi_off=pi_used))
                pi_used += npat
                continue
        denses.append(dict(gb=b, geo=geo))
    pi_tot = pi_used

    # ---- degenerate global data (same for all cores) ----
    sg_blocks, sr_blocks = [], []
    for d in degs:
        tables, ptab = d['tables'], d['ptab']
        npat = ptab.shape[0]
        slabs = _deg_slabs(tables)
        d['nslab'] = len(slabs)
        for rows, sigs in slabs:
            slab = np.zeros((128, C), dtype=np.float16)
            rl = sorted(rows, key=rows.get)
            if rl:
                slab[:len(rl)] = xT16[d['gb']][np.array(rl)]
            sigset = set(sigs)
            SR = np.zeros((128, 9 * npat), dtype=np.float32)
            for pi in range(npat):
                for tap in range(9):
                    si = ptab[pi, tap]
                    if si >= 0 and si in sigset:
                        for rr, ww in tables[si]:
                            SR[rows[rr], tap * npat + pi] += ww
            sg_blocks.append(slab)
            sr_blocks.append(SR.astype(np.float16))
    sg = (np.concatenate(sg_blocks, axis=1) if sg_blocks
          else np.zeros((128, 0), np.float16))
    sr = (np.concatenate(sr_blocks, axis=1) if sr_blocks
          else np.zeros((128, 0), np.float16))

    # ---- dense per-core blocks ----
    # block = 2 image rows 8r-2+2*blk, +1 of a dense batch (halo included;
    # out-of-image rows get zero columns); all 3 samples share its slabs
    core_units = []   # [core][block] -> list of (rowmap, S)
    for r in range(NCORE):
        units = []
        for d in denses:
            for blk in range(NBLK):
                row0 = 8 * r - 2 + 2 * blk
                pix = []
                for lr in range(2):
                    row = row0 + lr
                    pix += [row * W + cx if 0 <= row < H else -1
                            for cx in range(W)]
                units.append(_dense_block_slabs(pix, d['geo']))
        core_units.append(units)
    nunits = len(core_units[0])
    unit_nslab = [max(1, max(len(core_units[r][u]) for r in range(NCORE)))
                  for u in range(nunits)]

    ui = 0
    for d in denses:
        d['unit_nslab'] = tuple(unit_nslab[ui:ui + NBLK])
        ui += NBLK

    # ---- weights ----
    Wl = np.zeros((128, 3 * 9 * C), dtype=np.float16)
    cw = np.asarray(conv_w, np.float32)
    cb = np.asarray(conv_b, np.float32)
    for kc, (koff, ksz) in enumerate(MB):
        for tap in range(9):
            dy, dx = tap // 3, tap % 3
            Wl[0:ksz, kc * 9 * C + tap * C: kc * 9 * C + tap * C + C] = \
                cw[:, koff:koff + ksz, dy, dx].T.astype(np.float16)
    bias = np.zeros((128, 3), dtype=np.float32)
    for mc, (moff, msz) in enumerate(MB):
        bias[0:msz, mc] = cb[moff:moff + msz]

    # ---- per-core in_maps ----
    ndeg, ndense = len(degs), len(denses)
    slots = [d['gb'] for d in degs] + [d['gb'] for d in denses]
    in_maps = []
    for r in range(NCORE):
        m = {"wl": Wl, "bias": bias}
        if ndeg:
            m["sg"] = sg
            m["sr"] = sr
            e = np.zeros((128, ndeg * STRIP * W), dtype=np.float16)
            for bi, d in enumerate(degs):
                pidx = d['pinv'].reshape(H, W)[8 * r: 8 * r + STRIP].ravel()
                e[d['pi_off'] + pidx,
                  bi * STRIP * W + np.arange(STRIP * W)] = 1.0
            m["e_mat"] = e
        if ndense:
            sds, sss = [], []
            for u in range(nunits):
                slabs = core_units[r][u]
                di = u // NBLK
                gb = denses[di]['gb']
                for j in range(unit_nslab[u]):
                    slab = np.zeros((128, C), dtype=np.float16)
                    S = np.zeros((128, 128), dtype=np.float16)
                    if j < len(slabs):
                        rows, Sf = slabs[j]
                        rl = sorted(rows, key=rows.get)
                        if rl:
                            slab[:len(rl)] = xT16[gb][np.array(rl)]
                        S = Sf.astype(np.float16)
                    sds.append(slab)
                    sss.append(S)
            m["sd"] = np.concatenate(sds, axis=1)
            m["ss"] = np.concatenate(sss, axis=1)
        in_maps.append(m)

    struct = (pi_tot,
              tuple((d['gb'], d['ptab'].shape[0], d['nslab']) for d in degs),
              tuple((d['gb'], d['unit_nslab']) for d in denses))
    return in_maps, struct, slots


# ------------------------------------------------------------- bass program

_NC_CACHE = {}


def build_program(reps, struct):
    key = (reps, struct)
    if key in _NC_CACHE:
        return _NC_CACHE[key]
    import concourse.bacc as bacc
    import concourse.mybir as mybir
    from concourse.tile import TileContext

    fp16 = mybir.dt.float16
    f32 = mybir.dt.float32
    pi_tot, degs, denses = struct
    ndeg, ndense = len(degs), len(denses)
    NB = ndeg + ndense
    NSG = sum(ns for _, _, ns in degs)
    SRC = sum(9 * npat * ns for _, npat, ns in degs)
    NSLAB = sum(sum(us) for _, us in denses)
    SW = STRIP * W   # 512 pixels per strip

    nc = bacc.Bacc(target_bir_lowering=False)
    wl_d = nc.dram_tensor("wl", [128, 3 * 9 * C], fp16, kind="ExternalInput")
    bias_d = nc.dram_tensor("bias", [128, 3], f32, kind="ExternalInput")
    if ndeg:
        sg_d = nc.dram_tensor("sg", [128, NSG * C], fp16, kind="ExternalInput")
        sr_d = nc.dram_tensor("sr", [128, SRC], fp16, kind="ExternalInput")
        e_d = nc.dram_tensor("e_mat", [128, ndeg * SW], fp16,
                             kind="ExternalInput")
    if ndense:
        sd_d = nc.dram_tensor("sd", [128, NSLAB * C], fp16,
                              kind="ExternalInput")
        ss_d = nc.dram_tensor("ss", [128, NSLAB * 128], fp16,
                              kind="ExternalInput")
    out_d = nc.dram_tensor("out", [128, NB * 3 * SW], fp16,
                           kind="ExternalOutput")

    with TileContext(nc) as tc:
        with tc.tile_pool(name="const", bufs=1) as constp:
            wl = constp.tile([128, 3 * 9 * C], fp16)
            nc.sync.dma_start(out=wl[:], in_=wl_d[:])
            bias_t = constp.tile([128, 3], f32)
            nc.sync.dma_start(out=bias_t[:], in_=bias_d[:])

            def body(_it):
                with tc.tile_pool(name="inp", bufs=2) as inp, \
                     tc.tile_pool(name="sdp", bufs=1) as sdp, \
                     tc.tile_pool(name="ssp", bufs=1) as ssp, \
                     tc.tile_pool(name="work", bufs=1) as work, \
                     tc.tile_pool(name="smpp", bufs=2) as smpp, \
                     tc.tile_pool(name="outp", bufs=2) as outp, \
                     tc.tile_pool(name="psA", bufs=3, space="PSUM") as psA, \
                     tc.tile_pool(name="psT", bufs=1, space="PSUM") as psT, \
                     tc.tile_pool(name="psB", bufs=4, space="PSUM") as psB:

                    out_sb = outp.tile([128, NB, 3, SW], fp16, name="out_sb")
                    # mc=2 has only 64 valid channel partitions; zero the rest
                    # so the out DMA never reads uninitialized SBUF
                    nc.gpsimd.memset(out_sb[64:128, :, 2:3, :], 0.0)

                    # ---------- input DMAs ----------
                    if ndeg:
                        sg = inp.tile([128, NSG * C], fp16, name="sg",
                                      tag="sg")
                        nc.sync.dma_start(out=sg[:], in_=sg_d[:])
                        sr = inp.tile([128, SRC], fp16, name="sr", tag="sr")
                        nc.sync.dma_start(out=sr[:], in_=sr_d[:])
                        e = inp.tile([128, ndeg * SW], fp16, name="e",
                                     tag="e")
                        nc.sync.dma_start(out=e[:], in_=e_d[:])
                    sd_tiles, ss_tiles = [], []
                    if ndense:
                        # chunk slab streams per (dense batch, block)
                        off = 0
                        for di, (_, us) in enumerate(denses):
                            for blk in range(NBLK):
                                nsl = us[blk]
                                sdt = sdp.tile([128, nsl * C], fp16,
                                               tag=f"sd{blk}")
                                nc.sync.dma_start(
                                    out=sdt[:],
                                    in_=sd_d[:, off * C:(off + nsl) * C])
                                sst = ssp.tile([128, nsl * 128], fp16,
                                               tag=f"ss{blk}")
                                nc.sync.dma_start(
                                    out=sst[:],
                                    in_=ss_d[:, off * 128:(off + nsl) * 128])
                                sd_tiles.append(sdt)
                                ss_tiles.append(sst)
                                off += nsl

                    # ---------- degenerate path: R ----------
                    if ndeg:
                        rsb = work.tile([128, 3, 9, pi_tot], fp16, name="rsb")
                        sgo, sro, poff = 0, 0, 0
                        for bi, (_, npat, nsl) in enumerate(degs):
                            ps_r = psA.tile([128, 512], f32, name=f"psr{bi}",
                                            tag="psA")
                            for kc, (koff, ksz) in enumerate(MB):
                                for j in range(nsl):
                                    nc.tensor.matmul(
                                        ps_r[0:ksz, kc * 9 * npat:
                                             (kc + 1) * 9 * npat],
                                        sg[:, sgo + j * C + koff:
                                           sgo + j * C + koff + ksz],
                                        sr[:, sro + j * 9 * npat:
                                           sro + (j + 1) * 9 * npat],
                                        start=(j == 0), stop=(j == nsl - 1))
                            psv = ps_r[:, 0:27 * npat].rearrange(
                                "p (k t q) -> p k t q", k=3, t=9)
                            nc.vector.tensor_copy(
                                rsb[0:128, 0:2, :, poff:poff + npat],
                                psv[0:128, 0:2, :, :])
                            nc.vector.tensor_copy(
                                rsb[0:64, 2:3, :, poff:poff + npat],
                                psv[0:64, 2:3, :, :])
                            sgo += nsl * C
                            sro += nsl * 9 * npat
                            poff += npat

                    # ---------- dense sampling (first half) ----------
                    smps = []
                    if ndense:
                        for di in range(ndense):
                            smp = smpp.tile([128, 3, 2 * NBLK, 66], fp16,
                                            name=f"smp{di}", tag=f"smp{di}")
                            nc.gpsimd.memset(smp[:, :, :, 0:1], 0.0)
                            nc.gpsimd.memset(smp[:, :, :, 65:66], 0.0)
                            smps.append(smp)

                    def dense_block(di, blk):
                        _, us = denses[di]
                        smp = smps[di]
                        sdt = sd_tiles[di * NBLK + blk]
                        sst = ss_tiles[di * NBLK + blk]
                        ps = psA.tile([128, 512], f32,
                                      name=f"psb{di}_{blk}", tag="psA")
                        nsl = us[blk]
                        for kc, (koff, ksz) in enumerate(MB):
                            for k in range(nsl):
                                nc.tensor.matmul(
                                    ps[0:ksz, kc * 128:(kc + 1) * 128],
                                    sdt[:, k * C + koff:
                                        k * C + koff + ksz],
                                    sst[:, k * 128:(k + 1) * 128],
                                    start=(k == 0), stop=(k == nsl - 1))
                        psv = ps[:, 0:384].rearrange("p (k r c) -> p k r c",
                                                     k=3, r=2)
                        nc.vector.tensor_copy(
                            smp[0:128, 0:2, 2 * blk:2 * blk + 2, 1:65],
                            psv[0:128, 0:2, :, :])
                        nc.vector.tensor_copy(
                            smp[0:64, 2:3, 2 * blk:2 * blk + 2, 1:65],
                            psv[0:64, 2:3, :, :])

                    if ndense:
                        for di in range(ndense):
                            for blk in range(3):
                                dense_block(di, blk)

                    # ---------- degenerate path: T ----------
                    # bias + relu commute with the per-pixel column selection,
                    # so they are applied after the expansion matmul instead
                    if ndeg:
                        ps_t = psT.tile([128, 512], f32, name="ps_t")
                        k = 0
                        for kc, (koff, ksz) in enumerate(MB):
                            for tap in range(9):
                                nc.tensor.matmul(
                                    ps_t[0:pi_tot, 0:C],
                                    rsb[0:ksz, kc, tap, :],
                                    wl[0:ksz, kc * 9 * C + tap * C:
                                       kc * 9 * C + tap * C + C],
                                    start=(k == 0), stop=(k == 26))
                                k += 1
                        tsb = work.tile([128, C], fp16, name="tsb")
                        nc.scalar.copy(tsb[0:pi_tot, :], ps_t[0:pi_tot, 0:C])

                    # ---------- dense sampling (second half) ----------
                    if ndense:
                        for di in range(ndense):
                            for blk in range(3, NBLK):
                                dense_block(di, blk)

                    # ---------- expansion + dense conv, interleaved per mc ----
                    # the conv matmul groups run on PE while the expansion
                    # PSUM drains complete on DVE/ACT
                    for mc, (moff, msz) in enumerate(MB):
                        if ndeg:
                            for bi in range(ndeg):
                                ps_e = psB.tile([128, 512], f32,
                                                name=f"pse{mc}_{bi}",
                                                tag="psB")
                                nc.tensor.matmul(
                                    ps_e[0:msz, :],
                                    tsb[0:pi_tot, moff:moff + msz],
                                    e[0:pi_tot, bi * SW:(bi + 1) * SW],
                                    start=True, stop=True)
                                dst = out_sb[0:msz, bi, mc, :]
                                if bi % 2 == 0:
                                    nc.vector.tensor_scalar(
                                        dst, ps_e[0:msz, :],
                                        bias_t[0:msz, mc:mc + 1], 0.0,
                                        mybir.AluOpType.add,
                                        mybir.AluOpType.max)
                                else:
                                    nc.scalar.activation(
                                        dst, ps_e[0:msz, :],
                                        mybir.ActivationFunctionType.Relu,
                                        bias=bias_t[0:msz, mc:mc + 1])
                        for di in range(ndense):
                            smp = smps[di]
                            ps_c = psB.tile([128, 512], f32,
                                            name=f"psc{di}_{mc}", tag="psB")
                            k = 0
                            for kc, (koff, ksz) in enumerate(MB):
                                for tap in range(9):
                                    dy, dx = tap // 3, tap % 3
                                    nc.tensor.matmul(
                                        ps_c[0:msz, :],
                                        wl[0:ksz,
                                           kc * 9 * C + tap * C + moff:
                                           kc * 9 * C + tap * C + moff + msz],
                                        smp[0:ksz, kc, 1 + dy:9 + dy,
                                            dx:dx + 64],
                                        start=(k == 0), stop=(k == 26))
                                    k += 1
                            nc.scalar.activation(
                                out_sb[0:msz, ndeg + di, mc, :],
                                ps_c[0:msz, :],
                                mybir.ActivationFunctionType.Relu,
                                bias=bias_t[0:msz, mc:mc + 1])
                    for si in range(NB):
                        nc.sync.dma_start(
                            out=out_d[:, si * 3 * SW:(si + 1) * 3 * SW],
                            in_=out_sb[:, si, :, :])

            if reps == 1:
                body(0)
            else:
                with tc.For_i(0, reps, 1) as it:
                    body(it)

    nc.finalize()
    _NC_CACHE[key] = nc
    return nc


# ---------------------------------------------------------------- interface


def make_in_maps(x, source_intrinsics, target_intrinsics, source_pose,
                 target_pose, conv_w, conv_b):
    return make_plan(x, source_intrinsics, target_intrinsics, source_pose,
                     target_pose, conv_w, conv_b)


def assemble(results, slots):
    """results: list of per-core {"out": [128, NB*3*SW]} -> [B, C, H, W]."""
    out = np.zeros((B, C, H, W), dtype=np.float32)
    NBl = len(slots)
    for r in range(NCORE):
        o = np.asarray(results[r]["out"]).reshape(128, NBl, 3, STRIP, W)
        for si, gb in enumerate(slots):
            for mc, (moff, msz) in enumerate(MB):
                out[gb, moff:moff + msz, 8 * r: 8 * r + STRIP, :] = \
                    o[0:msz, si, mc].astype(np.float32)
    return out


def kernel(x, source_intrinsics, target_intrinsics, source_pose,
           target_pose, conv_w, conv_b, _reps=1):
    from concourse.bass_utils import run_bass_kernel_spmd
    in_maps, struct, slots = make_in_maps(
        x, source_intrinsics, target_intrinsics, source_pose,
        target_pose, conv_w, conv_b)
    nc = build_program(_reps, struct)
    res = run_bass_kernel_spmd(nc, in_maps, list(range(NCORE)))
    return assemble(res.results, slots)


# revision 26
# speedup vs baseline: 7.2964x; 1.1982x over previous
"""Trainium2 Bass kernel for nn_EpipolarWarpOperator (B=8, C=320, H=W=64, S=3).

Sharding: spatial — every core computes an 8-row horizontal strip of all 8
batch outputs (the work of a batch is dominated by the 3x3 conv, which is
strip-separable; the epipolar sampling is handled per strip with halo).

Host analysis classifies each batch by its epipolar sampling map:

* pattern batches: the per-pixel bilinear sampling signature map has few
  distinct values (the epipolar lines clip to a handful of source locations),
  so the sampled image is piecewise constant and each output column is one of
  npat distinct "pattern" columns (3x3 signature neighborhoods incl. the
  zero-pad border). Per batch the device computes R[k, tap, pi] (replicated
  sampled columns) from a host-gathered slab via tiny matmuls,
  T^T[pi, m] = sum_{tap,kc} R_tap.T @ W_tap (+bias via a ones-row), relu on
  ACT, then expands out = T^T.T @ E with a per-core 0/1 selection matrix E.
  All x-dependent math stays on device.

* dense batches: per (128-pixel block, sample) the host pre-gathers the
  <=128 distinct bilinear corner pixels into a slab [128, 320] plus a
  sampling matrix S [128, 128]; the device runs swapped-operand matmuls
  slab.T @ S accumulating all samples (and sub-slabs) into PSUM, directly in
  channel-major order, then a 27-matmul 3x3 conv over the strip (+halo row)
  with bias+relu on ACT.
"""

import numpy as np

B, C, H, W = 8, 320, 64, 64
SN = 3
HW = H * W
STRIP = 8             # output rows per core
NCORE = 8
MB = [(0, 128), (128, 128), (256, 64)]   # output/input channel chunking
PI_MAX = 126          # total pattern budget (K of the expansion matmul)
NPAT_MAX = 56         # per-batch pattern cap (9*npat*4B must fit a PSUM bank)
NSIG_MAX = 64
NBLK = 6              # 2-row sampling blocks per strip (incl. 2-row halo pad)

# ---------------------------------------------------------------- host: geometry


def _rodrigues_np(rv):
    theta = np.sqrt((rv * rv).sum())
    r = rv / max(theta, 1e-12)
    I = np.eye(3, dtype=np.float32)
    K = np.array([[0, -r[2], r[1]], [r[2], 0, -r[0]], [-r[1], r[0], 0]],
                 dtype=np.float32)
    R = np.cos(theta) * I + (1 - np.cos(theta)) * np.outer(r, r) + np.sin(theta) * K
    return I if theta < 1e-6 else R


def fundamental_np(Ks, Kt, ps, pt):
    Fs = []
    for b in range(Ks.shape[0]):
        Rs = _rodrigues_np(ps[b, :3].astype(np.float32))
        Rt = _rodrigues_np(pt[b, :3].astype(np.float32))
        ts_, tt_ = ps[b, 3:].astype(np.float32), pt[b, 3:].astype(np.float32)
        R_rel = Rs @ Rt.T
        t_rel = ts_ - R_rel @ tt_
        z = np.float32(0)
        skew = np.array([[z, -t_rel[2], t_rel[1]],
                         [t_rel[2], z, -t_rel[0]],
                         [-t_rel[1], t_rel[0], z]], dtype=np.float32)
        E = skew @ R_rel
        inv_Ks = np.linalg.inv(Ks[b].astype(np.float32))
        inv_Kt = np.linalg.inv(Kt[b].astype(np.float32))
        Fs.append(inv_Kt.T @ E @ inv_Ks)
    return np.stack(Fs).astype(np.float32)


def geometry(F):
    k = np.arange(HW)
    px = (k % W).astype(np.float32)
    py = (k // W).astype(np.float32)
    P = np.stack([px, py, np.ones_like(px)])
    lines = F.T.astype(np.float32) @ P
    a, b_, c = lines[0], lines[1], lines[2]
    W1, H1 = np.float32(W - 1), np.float32(H - 1)
    EPS = np.float32(1e-10)
    x1 = np.clip(-c / (a + EPS), 0.0, W1)
    x2 = np.clip(-(b_ * H1 + c) / (a + EPS), 0.0, W1)
    y1 = np.clip(-c / (b_ + EPS), 0.0, H1)
    y2 = np.clip(-(a * W1 + c) / (b_ + EPS), 0.0, H1)
    t = np.linspace(0.0, 1.0, SN, dtype=np.float32)
    sx = x1[:, None] * (1 - t) + x2[:, None] * t
    sy = y1[:, None] * (1 - t) + y2[:, None] * t
    x0 = np.floor(sx)
    y0 = np.floor(sy)
    wx = (sx - x0).astype(np.float32)
    wy = (sy - y0).astype(np.float32)
    x0i = np.clip(x0, 0, W - 1).astype(np.int32)
    y0i = np.clip(y0, 0, H - 1).astype(np.int32)
    return x0i, y0i, wx, wy


def _corners(geo, p, s):
    """4 bilinear (source pixel row, weight/3) pairs for pixel p, sample s."""
    x0i, y0i, wx, wy = geo
    y0 = int(y0i[p, s]); x0 = int(x0i[p, s])
    x1 = min(x0 + 1, W - 1); y1 = min(y0 + 1, H - 1)
    wxx = np.float32(wx[p, s]); wyy = np.float32(wy[p, s])
    third = np.float32(1.0 / 3.0)
    out = {}
    for ry, rx, ww in ((y0, x0, (1 - wxx) * (1 - wyy)),
                       (y0, x1, wxx * (1 - wyy)),
                       (y1, x0, (1 - wxx) * wyy),
                       (y1, x1, wxx * wyy)):
        rr = ry * W + rx
        out[rr] = out.get(rr, np.float32(0)) + ww * third
    return out


# ------------------------------------------------------------- host: classify


def classify(geo):
    """If the sampling map has <= NSIG_MAX distinct per-pixel signatures,
    return (siginv [HW], tables); else None."""
    x0i, y0i, wx, wy = geo
    key = np.concatenate([
        y0i.astype(np.float32), x0i.astype(np.float32), wx, wy], axis=1)
    kview = np.ascontiguousarray(key).view([('', key.dtype)] * key.shape[1]).ravel()
    uniq, first, inv = np.unique(kview, return_index=True, return_inverse=True)
    if len(uniq) > NSIG_MAX:
        return None
    tables = []
    for si in range(len(uniq)):
        p = int(first[si])
        contrib = {}
        for s in range(SN):
            for rr, ww in _corners(geo, p, s).items():
                contrib[rr] = contrib.get(rr, np.float32(0)) + ww
        tables.append(sorted(contrib.items()))
    return inv.astype(np.int32), tables


def patterns(siginv):
    """3x3 neighborhood patterns of the signature map (border = -1)."""
    simg = siginv.reshape(H, W)
    pad = np.pad(simg, 1, constant_values=-1)
    neigh = np.stack([pad[dy:dy + H, dx:dx + W].ravel()
                      for dy in range(3) for dx in range(3)], axis=1)
    uniq, pinv = np.unique(neigh, axis=0, return_inverse=True)
    return pinv.astype(np.int32), uniq


# ------------------------------------------------------------ host: build plan


def _deg_slabs(tables):
    """Assign signatures to slabs so each slab's source-pixel union <= 128.
    Returns list of (rowmap, sigs) with rowmap = {pixrow: slabrow}."""
    slabs = []
    rows, sigs = {}, []
    for si, tab in enumerate(tables):
        new = [rr for rr, _ in tab if rr not in rows]
        if len(rows) + len(new) > 128:
            slabs.append((rows, sigs))
            rows, sigs = {}, []
            new = [rr for rr, _ in tab]
        for rr in new:
            rows[rr] = len(rows)
        sigs.append(si)
    slabs.append((rows, sigs))
    return slabs


def _dense_block_slabs(pix_ids, geo):
    """Greedy split of a 128-pixel block's (pixel, sample) tokens into slabs
    of <=128 distinct corner pixels, tokens sorted by corner location so
    nearby tokens share slab rows. Returns list of (rowmap, S[128,128])."""
    x0i, y0i = geo[0], geo[1]
    toks = [(int(y0i[p, s]), int(x0i[p, s]), i, p, s)
            for i, p in enumerate(pix_ids) if p >= 0 for s in range(SN)]
    toks.sort()
    slabs = []
    rows = {}
    S = np.zeros((128, 128), dtype=np.float32)
    for _, _, i, p, s in toks:
        cs = _corners(geo, p, s)
        new = [rr for rr in cs if rr not in rows]
        if len(rows) + len(new) > 128:
            slabs.append((rows, S))
            rows = {}
            S = np.zeros((128, 128), dtype=np.float32)
            new = list(cs)
        for rr in new:
            rows[rr] = len(rows)
        for rr, ww in cs.items():
            S[rows[rr], i] += ww
    if rows or not slabs:
        slabs.append((rows, S))
    return slabs


def make_plan(x, source_intrinsics, target_intrinsics, source_pose,
              target_pose, conv_w, conv_b):
    Fs = fundamental_np(np.asarray(source_intrinsics, np.float32),
                        np.asarray(target_intrinsics, np.float32),
                        np.asarray(source_pose, np.float32),
                        np.asarray(target_pose, np.float32))
    x = np.asarray(x, np.float32)
    xT16 = [np.ascontiguousarray(x[b].reshape(C, HW).T).astype(np.float16)
            for b in range(B)]

    degs, denses = [], []
    pi_used = 0
    for b in range(B):
        geo = geometry(Fs[b])
        res = classify(geo)
        if res is not None:
            siginv, tables = res
            pinv, ptab = patterns(siginv)
            npat = ptab.shape[0]
            if npat <= NPAT_MAX and pi_used + npat <= PI_MAX:
                degs.append(dict(gb=b, siginv=siginv, tables=tables,
                                 pinv=pinv, ptab=ptab, pi_off=pi_used))
                pi_used += npat
                continue
        denses.append(dict(gb=b, geo=geo))
    pi_tot = pi_used

    # ---- degenerate global data (same for all cores) ----
    sg_blocks, sr_blocks = [], []
    for d in degs:
        tables, ptab = d['tables'], d['ptab']
        npat = ptab.shape[0]
        slabs = _deg_slabs(tables)
        d['nslab'] = len(slabs)
        for rows, sigs in slabs:
            slab = np.zeros((128, C), dtype=np.float16)
            rl = sorted(rows, key=rows.get)
            if rl:
                slab[:len(rl)] = xT16[d['gb']][np.array(rl)]
            sigset = set(sigs)
            SR = np.zeros((128, 9 * npat), dtype=np.float32)
            for pi in range(npat):
                for tap in range(9):
                    si = ptab[pi, tap]
                    if si >= 0 and si in sigset:
                        for rr, ww in tables[si]:
                            SR[rows[rr], tap * npat + pi] += ww
            sg_blocks.append(slab)
            sr_blocks.append(SR.astype(np.float16))
    sg = (np.concatenate(sg_blocks, axis=1) if sg_blocks
          else np.zeros((128, 0), np.float16))
    sr = (np.concatenate(sr_blocks, axis=1) if sr_blocks
          else np.zeros((128, 0), np.float16))

    # ---- dense per-core blocks ----
    # block = 2 image rows 8r-2+2*blk, +1 of a dense batch (halo included;
    # out-of-image rows get zero columns); all 3 samples share its slabs
    core_units = []   # [core][block] -> list of (rowmap, S)
    for r in range(NCORE):
        units = []
        for d in denses:
            for blk in range(NBLK):
                row0 = 8 * r - 2 + 2 * blk
                pix = []
                for lr in range(2):
                    row = row0 + lr
                    pix += [row * W + cx if 0 <= row < H else -1
                            for cx in range(W)]
                units.append(_dense_block_slabs(pix, d['geo']))
        core_units.append(units)
    nunits = len(core_units[0])
    unit_nslab = [max(1, max(len(core_units[r][u]) for r in range(NCORE)))
                  for u in range(nunits)]

    ui = 0
    for d in denses:
        d['unit_nslab'] = tuple(unit_nslab[ui:ui + NBLK])
        ui += NBLK

    # ---- weights ----
    Wl = np.zeros((128, 3 * 9 * C), dtype=np.float16)
    cw = np.asarray(conv_w, np.float32)
    cb = np.asarray(conv_b, np.float32)
    for kc, (koff, ksz) in enumerate(MB):
        for tap in range(9):
            dy, dx = tap // 3, tap % 3
            Wl[0:ksz, kc * 9 * C + tap * C: kc * 9 * C + tap * C + C] = \
                cw[:, koff:koff + ksz, dy, dx].T.astype(np.float16)
    bias = np.zeros((128, 3), dtype=np.float32)
    for mc, (moff, msz) in enumerate(MB):
        bias[0:msz, mc] = cb[moff:moff + msz]

    # ---- per-core in_maps ----
    ndeg, ndense = len(degs), len(denses)
    slots = [d['gb'] for d in degs] + [d['gb'] for d in denses]
    in_maps = []
    for r in range(NCORE):
        m = {"wl": Wl, "bias": bias}
        if ndeg:
            m["sg"] = sg
            m["sr"] = sr
            e = np.zeros((128, ndeg * STRIP * W), dtype=np.float16)
            for bi, d in enumerate(degs):
                pidx = d['pinv'].reshape(H, W)[8 * r: 8 * r + STRIP].ravel()
                e[d['pi_off'] + pidx,
                  bi * STRIP * W + np.arange(STRIP * W)] = 1.0
            m["e_mat"] = e
        if ndense:
            sds, sss = [], []
            for u in range(nunits):
                slabs = core_units[r][u]
                di = u // NBLK
                gb = denses[di]['gb']
                for j in range(unit_nslab[u]):
                    slab = np.zeros((128, C), dtype=np.float16)
                    S = np.zeros((128, 128), dtype=np.float16)
                    if j < len(slabs):
                        rows, Sf = slabs[j]
                        rl = sorted(rows, key=rows.get)
                        if rl:
                            slab[:len(rl)] = xT16[gb][np.array(rl)]
                        S = Sf.astype(np.float16)
                    sds.append(slab)
                    sss.append(S)
            m["sd"] = np.concatenate(sds, axis=1)
            m["ss"] = np.concatenate(sss, axis=1)
        in_maps.append(m)

    struct = (pi_tot,
              tuple((d['gb'], d['ptab'].shape[0], d['nslab']) for d in degs),
              tuple((d['gb'], d['unit_nslab']) for d in denses))
    return in_maps, struct, slots


# ------------------------------------------------------------- bass program

_NC_CACHE = {}


def build_program(reps, struct):
    key = (reps, struct)
    if key in _NC_CACHE:
        return _NC_CACHE[key]
    import concourse.bacc as bacc
    import concourse.mybir as mybir
    from concourse.tile import TileContext

    fp16 = mybir.dt.float16
    f32 = mybir.dt.float32
    pi_tot, degs, denses = struct
    ndeg, ndense = len(degs), len(denses)
    NB = ndeg + ndense
    NSG = sum(ns for _, _, ns in degs)
    SRC = sum(9 * npat * ns for _, npat, ns in degs)
    NSLAB = sum(sum(us) for _, us in denses)
    SW = STRIP * W   # 512 pixels per strip

    nc = bacc.Bacc(target_bir_lowering=False)
    wl_d = nc.dram_tensor("wl", [128, 3 * 9 * C], fp16, kind="ExternalInput")
    bias_d = nc.dram_tensor("bias", [128, 3], f32, kind="ExternalInput")
    if ndeg:
        sg_d = nc.dram_tensor("sg", [128, NSG * C], fp16, kind="ExternalInput")
        sr_d = nc.dram_tensor("sr", [128, SRC], fp16, kind="ExternalInput")
        e_d = nc.dram_tensor("e_mat", [128, ndeg * SW], fp16,
                             kind="ExternalInput")
    if ndense:
        sd_d = nc.dram_tensor("sd", [128, NSLAB * C], fp16,
                              kind="ExternalInput")
        ss_d = nc.dram_tensor("ss", [128, NSLAB * 128], fp16,
                              kind="ExternalInput")
    out_d = nc.dram_tensor("out", [128, NB * 3 * SW], fp16,
                           kind="ExternalOutput")

    with TileContext(nc) as tc:
        with tc.tile_pool(name="const", bufs=1) as constp, \
             tc.tile_pool(name="inp", bufs=2) as inp, \
             tc.tile_pool(name="sdp", bufs=2) as sdp, \
             tc.tile_pool(name="ssp", bufs=2) as ssp, \
             tc.tile_pool(name="work", bufs=2) as work, \
             tc.tile_pool(name="smpp", bufs=2) as smpp, \
             tc.tile_pool(name="outp", bufs=2) as outp, \
             tc.tile_pool(name="psA", bufs=3, space="PSUM") as psA, \
             tc.tile_pool(name="psT", bufs=1, space="PSUM") as psT, \
             tc.tile_pool(name="psB", bufs=4, space="PSUM") as psB:
            wl = constp.tile([128, 3 * 9 * C], fp16)
            nc.sync.dma_start(out=wl[:], in_=wl_d[:])
            bias_t = constp.tile([128, 3], f32)
            nc.sync.dma_start(out=bias_t[:], in_=bias_d[:])

            def body(_it):
                if True:
                    out_sb = outp.tile([128, NB, 3, SW], fp16, name="out_sb",
                                       tag="out_sb")
                    # mc=2 has only 64 valid channel partitions; zero the rest
                    # so the out DMA never reads uninitialized SBUF
                    nc.gpsimd.memset(out_sb[64:128, :, 2:3, :], 0.0)

                    # ---------- input DMAs ----------
                    if ndeg:
                        sg = inp.tile([128, NSG * C], fp16, name="sg",
                                      tag="sg")
                        nc.sync.dma_start(out=sg[:], in_=sg_d[:])
                        sr = inp.tile([128, SRC], fp16, name="sr", tag="sr")
                        nc.sync.dma_start(out=sr[:], in_=sr_d[:])
                        e = inp.tile([128, ndeg * SW], fp16, name="e",
                                     tag="e")
                        nc.sync.dma_start(out=e[:], in_=e_d[:])
                    sd_tiles, ss_tiles = [], []
                    if ndense:
                        # chunk slab streams per (dense batch, block)
                        off = 0
                        for di, (_, us) in enumerate(denses):
                            for blk in range(NBLK):
                                nsl = us[blk]
                                sdt = sdp.tile([128, nsl * C], fp16,
                                               tag=f"sd{blk}")
                                nc.sync.dma_start(
                                    out=sdt[:],
                                    in_=sd_d[:, off * C:(off + nsl) * C])
                                sst = ssp.tile([128, nsl * 128], fp16,
                                               tag=f"ss{blk}")
                                nc.sync.dma_start(
                                    out=sst[:],
                                    in_=ss_d[:, off * 128:(off + nsl) * 128])
                                sd_tiles.append(sdt)
                                ss_tiles.append(sst)
                                off += nsl

                    # ---------- degenerate path: R ----------
                    if ndeg:
                        rsb = work.tile([128, 3, 9, pi_tot], fp16, name="rsb",
                                        tag="rsb")
                        sgo, sro, poff = 0, 0, 0
                        for bi, (_, npat, nsl) in enumerate(degs):
                            ps_r = psA.tile([128, 512], f32, name=f"psr{bi}",
                                            tag="psA")
                            for kc, (koff, ksz) in enumerate(MB):
                                for j in range(nsl):
                                    nc.tensor.matmul(
                                        ps_r[0:ksz, kc * 9 * npat:
                                             (kc + 1) * 9 * npat],
                                        sg[:, sgo + j * C + koff:
                                           sgo + j * C + koff + ksz],
                                        sr[:, sro + j * 9 * npat:
                                           sro + (j + 1) * 9 * npat],
                                        start=(j == 0), stop=(j == nsl - 1))
                            psv = ps_r[:, 0:27 * npat].rearrange(
                                "p (k t q) -> p k t q", k=3, t=9)
                            nc.vector.tensor_copy(
                                rsb[0:128, 0:2, :, poff:poff + npat],
                                psv[0:128, 0:2, :, :])
                            nc.vector.tensor_copy(
                                rsb[0:64, 2:3, :, poff:poff + npat],
                                psv[0:64, 2:3, :, :])
                            sgo += nsl * C
                            sro += nsl * 9 * npat
                            poff += npat

                    # ---------- dense sampling (first half) ----------
                    smps = []
                    if ndense:
                        for di in range(ndense):
                            smp = smpp.tile([128, 3, 2 * NBLK, 66], fp16,
                                            name=f"smp{di}", tag=f"smp{di}")
                            nc.gpsimd.memset(smp[:, :, :, 0:1], 0.0)
                            nc.gpsimd.memset(smp[:, :, :, 65:66], 0.0)
                            smps.append(smp)

                    def dense_block(di, blk):
                        _, us = denses[di]
                        smp = smps[di]
                        sdt = sd_tiles[di * NBLK + blk]
                        sst = ss_tiles[di * NBLK + blk]
                        ps = psA.tile([128, 512], f32,
                                      name=f"psb{di}_{blk}", tag="psA")
                        nsl = us[blk]
                        for kc, (koff, ksz) in enumerate(MB):
                            for k in range(nsl):
                                nc.tensor.matmul(
                                    ps[0:ksz, kc * 128:(kc + 1) * 128],
                                    sdt[:, k * C + koff:
                                        k * C + koff + ksz],
                                    sst[:, k * 128:(k + 1) * 128],
                                    start=(k == 0), stop=(k == nsl - 1))
                        psv = ps[:, 0:384].rearrange("p (k r c) -> p k r c",
                                                     k=3, r=2)
                        nc.vector.tensor_copy(
                            smp[0:128, 0:2, 2 * blk:2 * blk + 2, 1:65],
                            psv[0:128, 0:2, :, :])
                        nc.vector.tensor_copy(
                            smp[0:64, 2:3, 2 * blk:2 * blk + 2, 1:65],
                            psv[0:64, 2:3, :, :])

                    if ndense:
                        for di in range(ndense):
                            for blk in range(3):
                                dense_block(di, blk)

                    # ---------- degenerate path: T ----------
                    # bias + relu commute with the per-pixel column selection,
                    # so they are applied after the expansion matmul instead
                    if ndeg:
                        ps_t = psT.tile([128, 512], f32, name="ps_t")
                        k = 0
                        for kc, (koff, ksz) in enumerate(MB):
                            for tap in range(9):
                                nc.tensor.matmul(
                                    ps_t[0:pi_tot, 0:C],
                                    rsb[0:ksz, kc, tap, :],
                                    wl[0:ksz, kc * 9 * C + tap * C:
                                       kc * 9 * C + tap * C + C],
                                    start=(k == 0), stop=(k == 26))
                                k += 1
                        tsb = work.tile([128, C], fp16, name="tsb", tag="tsb")
                        nc.scalar.copy(tsb[0:pi_tot, :], ps_t[0:pi_tot, 0:C])

                    # ---------- dense sampling (second half) ----------
                    if ndense:
                        for di in range(ndense):
                            for blk in range(3, NBLK):
                                dense_block(di, blk)

                    # ---------- expansion + dense conv, interleaved per mc ----
                    # the conv matmul groups run on PE while the expansion
                    # PSUM drains complete on DVE/ACT
                    for mc, (moff, msz) in enumerate(MB):
                        if ndeg:
                            for bi in range(ndeg):
                                ps_e = psB.tile([128, 512], f32,
                                                name=f"pse{mc}_{bi}",
                                                tag="psB")
                                nc.tensor.matmul(
                                    ps_e[0:msz, :],
                                    tsb[0:pi_tot, moff:moff + msz],
                                    e[0:pi_tot, bi * SW:(bi + 1) * SW],
                                    start=True, stop=True)
                                dst = out_sb[0:msz, bi, mc, :]
                                if bi % 2 == 0:
                                    nc.vector.tensor_scalar(
                                        dst, ps_e[0:msz, :],
                                        bias_t[0:msz, mc:mc + 1], 0.0,
                                        mybir.AluOpType.add,
                                        mybir.AluOpType.max)
                                else:
                                    nc.scalar.activation(
                                        dst, ps_e[0:msz, :],
                                        mybir.ActivationFunctionType.Relu,
                                        bias=bias_t[0:msz, mc:mc + 1])
                        for di in range(ndense):
                            smp = smps[di]
                            ps_c = psB.tile([128, 512], f32,
                                            name=f"psc{di}_{mc}", tag="psB")
                            k = 0
                            for kc, (koff, ksz) in enumerate(MB):
                                for tap in range(9):
                                    dy, dx = tap // 3, tap % 3
                                    nc.tensor.matmul(
                                        ps_c[0:msz, :],
                                        wl[0:ksz,
                                           kc * 9 * C + tap * C + moff:
                                           kc * 9 * C + tap * C + moff + msz],
                                        smp[0:ksz, kc, 1 + dy:9 + dy,
                                            dx:dx + 64],
                                        start=(k == 0), stop=(k == 26))
                                    k += 1
                            nc.scalar.activation(
                                out_sb[0:msz, ndeg + di, mc, :],
                                ps_c[0:msz, :],
                                mybir.ActivationFunctionType.Relu,
                                bias=bias_t[0:msz, mc:mc + 1])
                    for si in range(NB):
                        nc.sync.dma_start(
                            out=out_d[:, si * 3 * SW:(si + 1) * 3 * SW],
                            in_=out_sb[:, si, :, :])

            if reps == 1:
                body(0)
            else:
                # manual 3x unroll inside the hardware loop: consecutive
                # repetitions overlap through the double-buffered pools and
                # the For_i all-engine barrier is amortized over 3 reps
                U = 3
                n_loop = reps // U
                with tc.For_i(0, n_loop, 1) as it:
                    for u in range(U):
                        body(u)
                for u in range(reps - n_loop * U):
                    body(u)

    nc.finalize()
    _NC_CACHE[key] = nc
    return nc


# ---------------------------------------------------------------- interface


def make_in_maps(x, source_intrinsics, target_intrinsics, source_pose,
                 target_pose, conv_w, conv_b):
    return make_plan(x, source_intrinsics, target_intrinsics, source_pose,
                     target_pose, conv_w, conv_b)


def assemble(results, slots):
    """results: list of per-core {"out": [128, NB*3*SW]} -> [B, C, H, W]."""
    out = np.zeros((B, C, H, W), dtype=np.float32)
    NBl = len(slots)
    for r in range(NCORE):
        o = np.asarray(results[r]["out"]).reshape(128, NBl, 3, STRIP, W)
        for si, gb in enumerate(slots):
            for mc, (moff, msz) in enumerate(MB):
                out[gb, moff:moff + msz, 8 * r: 8 * r + STRIP, :] = \
                    o[0:msz, si, mc].astype(np.float32)
    return out


def kernel(x, source_intrinsics, target_intrinsics, source_pose,
           target_pose, conv_w, conv_b, _reps=1):
    from concourse.bass_utils import run_bass_kernel_spmd
    in_maps, struct, slots = make_in_maps(
        x, source_intrinsics, target_intrinsics, source_pose,
        target_pose, conv_w, conv_b)
    nc = build_program(_reps, struct)
    res = run_bass_kernel_spmd(nc, in_maps, list(range(NCORE)))
    return assemble(res.results, slots)
